# revision 30
# baseline (speedup 1.0000x reference)
"""Trainium2 Bass kernel for BioSphericalCKN1D (dense_cnn).

Computes, for x (32, 4096, 128), W (15, 128, 256), scale (1,1,1), bias (256):

    dot[b,t,f]  = sum_{k,c} x[b,t+k,c] * W[k,c,f]          (VALID conv, T_out = 4082)
    win[b,t]    = sum_{k,c} x[b,t+k,c]^2
    out[b,t,f]  = scale * dot / sqrt(win + 1e-7) + bias

Strategy:
  - Data-parallel over batch: 8 cores x 4 batches each; kernel/scale/bias replicated.
  - Per batch on-core:
      * load x in 1MB super-tiles, TensorE-transpose 128x128 tiles to xT [c, t]
      * ACT Square with fused row-sum accum -> per-t sum-of-squares S
      * sliding 15-window sums of S via 2 matmuls with constant band matrices
      * main conv: per 128-t tile, accumulate 15 float32r matmuls in PSUM
        (stationary = xT slice [c,128t], moving = W[k] [c,256f]) -> psum [t, f]
      * evacuate with ACT Copy scaled by r = scale/sqrt(win+eps) (per-partition),
        DVE add broadcast bias, batched contiguous DMA store.
  - Batch b+1's load/transpose/norm prologue is interleaved into batch b's
    main-matmul loop so the PE never idles (avoids HAM re-throttle).
"""

import os

import numpy as np

import concourse.bacc as bacc
import concourse.bass as bass
import concourse.mybir as mybir
import concourse.tile as tile
from concourse.bass_utils import run_bass_kernel_spmd

B, L, CIN, F, K = 32, 4096, 128, 256, 15
NCORES = 8
BP = B // NCORES          # batches per core
TOUT = L - K + 1          # 4082
NT = L // 128             # 32 row-tiles per batch
LG = 8                    # 128-row tiles per load super-tile (1 MB DMA)
SG = 4                    # j-tiles per store group (512 KB DMA)
EPS = 1e-7

# float32r = TF32-like PE mode: 1 cycle/row vs 4 for float32 (moving dim >= 256).
MM_DT = (
    mybir.dt.float32r
    if os.environ.get("KERNEL_MM_DT", "float32r") == "float32r"
    else mybir.dt.float32
)

_cache: dict = {}
last_results = None


def _build():
    nc = bacc.Bacc("TRN2", target_bir_lowering=False, debug=False, num_devices=NCORES)

    x_d = nc.dram_tensor("x", (BP, L, CIN), mybir.dt.float32, kind="ExternalInput")
    w_d = nc.dram_tensor("w", (K, CIN, F), mybir.dt.float32, kind="ExternalInput")
    scale_d = nc.dram_tensor("scale", (1, 1, 1), mybir.dt.float32, kind="ExternalInput")
    bias_d = nc.dram_tensor("bias", (F,), mybir.dt.float32, kind="ExternalInput")
    out_d = nc.dram_tensor("out", (BP, TOUT, F), mybir.dt.float32, kind="ExternalOutput")

    # Band matrices for the sliding-window sum: win[i*128+p] = sum_k sq[i*128+p+k]
    #   = (A.T @ S[:, i]) + (B.T @ S[:, i+1])   with A[q,p]=1 iff 0<=q-p<=K-1,
    #                                                B[q,p]=1 iff p-q>=128-(K-1)
    q = np.arange(128)[:, None]
    p = np.arange(128)[None, :]
    A_np = ((q - p >= 0) & (q - p <= K - 1)).astype(np.float32)
    B_np = (p - q >= 128 - (K - 1)).astype(np.float32)
    A_d = nc.inline_tensor(A_np, "bandA")
    B_d = nc.inline_tensor(B_np, "bandB")
    I_d = nc.inline_tensor(np.eye(128, dtype=np.float32), "ident")

    XT_COLS = L + 128  # main-mm lhsT slices read up to col 4109; zero-pad tail

    with tile.TileContext(nc) as tc:
        with (
            tc.tile_pool(name="const", bufs=1) as cpool,
            tc.tile_pool(name="xn", bufs=3) as xnpool,
            tc.tile_pool(name="sqs", bufs=3) as sqpool,
            tc.tile_pool(name="xt", bufs=2) as xtpool,
            tc.tile_pool(name="sums", bufs=2) as spool,
            tc.tile_pool(name="small", bufs=2) as smpool,
            tc.tile_pool(name="res", bufs=3) as respool,
            tc.tile_pool(name="pso", bufs=6, space=bass.MemorySpace.PSUM) as psopool,
            tc.tile_pool(name="pst", bufs=1, space=bass.MemorySpace.PSUM) as pstpool,
            tc.tile_pool(name="psw", bufs=1, space=bass.MemorySpace.PSUM) as pswpool,
        ):
            A_sb = cpool.tile([128, 128], mybir.dt.float32, tag="A")
            B_sb = cpool.tile([128, 128], mybir.dt.float32, tag="B")
            ident = cpool.tile([128, 128], mybir.dt.float32, tag="I")
            w_sb = cpool.tile([128, K, F], MM_DT, tag="W")
            bias_bc = cpool.tile([128, F], mybir.dt.float32, tag="bias")
            scale_col = cpool.tile([128, 1], mybir.dt.float32, tag="scale")
            eps_col = cpool.tile([128, 1], mybir.dt.float32, tag="eps")
            nc.vector.memset(eps_col[:], EPS)
            zeros128 = cpool.tile([128, 128], mybir.dt.float32, tag="zeros")
            nc.vector.memset(zeros128[:], 0.0)

            # identity/band consts first on the fast HWDGE queue (tiny, and
            # the very first transposes/win matmuls need them); W first on the
            # gpsimd queue with a casting DMA (fp32 -> float32r rounds, which
            # satisfies the verifier's fp32r-producer rule)
            nc.sync.dma_start(ident[:], I_d[:])
            nc.sync.dma_start(A_sb[:], A_d[:])
            nc.sync.dma_start(B_sb[:], B_d[:])
            # W split so batch-0's first mains track the arrival: first taps
            # via the fast HWDGE queue (stage + DVE cast), rest via casting
            # SWDGE chunks
            w_stage = cpool.tile([128, 4, F], mybir.dt.float32, tag="Wstage")
            nc.sync.dma_start(w_stage[:], w_d[0:4].transpose([1, 0, 2]))
            nc.vector.tensor_copy(w_sb[:, 0:4, :], w_stage[:])
            for k0, k1 in ((4, 8), (8, 12), (12, K)):
                nc.gpsimd.dma_start(
                    w_sb[:, k0:k1, :], w_d[k0:k1].transpose([1, 0, 2])
                )
            nc.gpsimd.dma_start(
                bias_bc[:], bias_d[:].unsqueeze(0).partition_broadcast(128).squeeze(1)
            )
            nc.gpsimd.dma_start(
                scale_col[:],
                scale_d[:].flatten().unsqueeze(0).partition_broadcast(128).squeeze(1),
            )

            # Per-batch persistent tiles, created lazily by the prologue.
            xts = {}
            Ss = {}
            Rs = {}

            def load_group(b, i0, n, engine=None):
                """DMA one super-tile (n 128-row tiles, starting at tile i0)."""
                xn = xnpool.tile([128, n, 128], mybir.dt.float32, tag="xn")
                # src: x[b, i0*128 + i*128 + p, c] -> dest (p, i, c)
                src = x_d[b, i0 * 128 : (i0 + n) * 128, :].rearrange(
                    "(i p) c -> p i c", p=128
                )
                (engine or nc.sync).dma_start(xn[:], src)
                return xn

            def prologue_item(b, i, xn_group):
                """Square+transpose 128-row tile i of batch b from its super-tile."""
                if b not in xts:
                    xts[b] = xtpool.tile([128, XT_COLS], MM_DT, tag="xt", name=f"xt{b}")
                    Ss[b] = spool.tile([128, NT + 1], mybir.dt.float32, tag="S", name=f"S{b}")
                    Rs[b] = smpool.tile([128, NT], mybir.dt.float32, tag="R", name=f"R{b}")
                    nc.vector.tensor_copy(xts[b][:, L:XT_COLS], zeros128[:])
                    nc.vector.memset(Ss[b][:, NT : NT + 1], 0.0)
                xt, S = xts[b], Ss[b]
                if xn_group.ndim == 3:
                    xn_slice = xn_group[:, i % LG, :]
                else:
                    xn_slice = xn_group
                sq = sqpool.tile([128, 128], mybir.dt.float32, tag="sq")
                nc.scalar.activation(
                    sq[:],
                    xn_slice,
                    mybir.ActivationFunctionType.Square,
                    accum_out=S[:, i : i + 1],
                )
                pst = pstpool.tile([128, 128], mybir.dt.float32, tag="pst")
                nc.tensor.transpose(pst[:], xn_slice, ident[:])
                nc.vector.tensor_copy(xt[:, i * 128 : (i + 1) * 128], pst[:])

            def win_r(b, c0=0, c1=NT):
                """r = scale/sqrt(win+eps) for S columns [c0, c1) of batch b."""
                S, R = Ss[b], Rs[b]
                n = c1 - c0
                winp = pswpool.tile([128, n], mybir.dt.float32, tag="win", name=f"win{b}_{c0}")
                nc.tensor.matmul(winp[:], A_sb[:], S[:, c0:c1], start=True, stop=False)
                nc.tensor.matmul(
                    winp[:], B_sb[:], S[:, c0 + 1 : c1 + 1], start=False, stop=True
                )
                sd = smpool.tile([128, n], mybir.dt.float32, tag="sd", name=f"sd{b}_{c0}")
                nc.scalar.activation(
                    sd[:], winp[:], mybir.ActivationFunctionType.Sqrt, bias=eps_col[:]
                )
                rin = smpool.tile([128, n], mybir.dt.float32, tag="rin", name=f"rin{b}_{c0}")
                nc.vector.reciprocal(rin[:], sd[:])
                nc.vector.tensor_scalar_mul(R[:, c0:c1], rin[:], scale_col[:])

            def b0_head(b):
                """Batch-0 cold start: first 9 tiles with graduated load groups
                across both HWDGE queues, then r for the first 8 out-tiles."""
                sizes = [1, 1, 2, 4, 4]
                engines = [nc.scalar, nc.sync, nc.scalar, nc.sync, nc.scalar]
                i0 = 0
                for g, n in enumerate(sizes):
                    xn_group = load_group(b, i0, n, engines[g])
                    for i in range(i0, i0 + n):
                        prologue_item(b, i, xn_group[:, i - i0 : i - i0 + 1, :].squeeze(1))
                        if i == 8:
                            win_r(b, 0, 8)
                    i0 += n

            def b0_tail_item(b, i, xn_holder):
                """Items 9..31 of batch 0, in load groups of 4 on alternating
                queues; win chunks as soon as their S columns are complete.
                NOTE: every R slice must be emitted before any main that reads
                it -- Tile tracks deps by trace order only."""
                if i % 4 == 0:
                    eng = nc.sync if (i // 4) % 2 == 0 else nc.scalar
                    xn_holder[0] = load_group(b, i, 4, eng)
                    xn_holder[1] = i
                prologue_item(
                    b, i, xn_holder[0][:, i - xn_holder[1] : i - xn_holder[1] + 1, :].squeeze(1)
                )
                if i in (16, 24):
                    win_r(b, i - 8, i)
                elif i == NT - 1:
                    win_r(b, 24, NT)

            def main_tile(b, j, ob_group):
                """15 accumulated matmuls + evacuation for 128-row out tile j."""
                xt, R = xts[b], Rs[b]
                po = psopool.tile([128, F], mybir.dt.float32, tag="po")
                for k in range(K):
                    nc.tensor.matmul(
                        po[:],
                        xt[:, j * 128 + k : j * 128 + k + 128],
                        w_sb[:, k, :],
                        start=(k == 0),
                        stop=(k == K - 1),
                    )
                ob_slice = ob_group[:, j % SG, :]
                nc.scalar.activation(
                    ob_slice,
                    po[:],
                    mybir.ActivationFunctionType.Copy,
                    scale=R[:, j : j + 1],
                )
                nc.vector.tensor_add(ob_slice, ob_slice, bias_bc[:])

            def store_group(b, g, ob_group):
                """Store SG evacuated j-tiles as one DMA (split if partial)."""
                j0 = g * SG
                t0 = j0 * 128
                full = min(SG, (TOUT - t0 + 127) // 128)
                # dest iterated as (p, jj, f) to match src tile dims
                last_rows = min(128, TOUT - (j0 + full - 1) * 128)
                if last_rows == 128:
                    dst = out_d[b, t0 : t0 + full * 128, :].rearrange(
                        "(jj p) f -> p jj f", p=128
                    )
                    nc.scalar.dma_start(dst, ob_group[:, 0:full, :])
                else:
                    if full > 1:
                        dst = out_d[b, t0 : t0 + (full - 1) * 128, :].rearrange(
                            "(jj p) f -> p jj f", p=128
                        )
                        nc.scalar.dma_start(dst, ob_group[:, 0 : full - 1, :])
                    tl = (j0 + full - 1) * 128
                    nc.scalar.dma_start(
                        out_d[b, tl : tl + last_rows, :],
                        ob_group[:last_rows, full - 1, :],
                    )

            # ---- schedule ----
            b0_head(0)
            xn_holder = [None, 0]
            for b in range(BP):
                nxt = b + 1
                ob_group = None
                xn_cur = None
                for j in range(NT):
                    if b == 0 and j < 20:
                        # finish batch 0's own prologue under its mains;
                        # emitted BEFORE the main so win chunks precede readers
                        b0_tail_item(0, j + 12, xn_holder)
                    if j % SG == 0:
                        ob_group = respool.tile([128, SG, F], mybir.dt.float32, tag="ob")
                    main_tile(b, j, ob_group)
                    if j % SG == SG - 1:
                        store_group(b, j // SG, ob_group)
                    # interleave next batch's prologue into this batch's mains
                    if nxt < BP:
                        if j % LG == 0:
                            xn_cur = load_group(nxt, j, LG)
                        prologue_item(nxt, j, xn_cur)
                        if j == NT - 1:
                            win_r(nxt)

    nc.compile()
    return nc


def kernel(**inputs: np.ndarray) -> np.ndarray:
    global last_results
    x = np.ascontiguousarray(np.asarray(inputs["inputs"], dtype=np.float32))
    w = np.ascontiguousarray(np.asarray(inputs["kernel"], dtype=np.float32))
    scale = np.ascontiguousarray(np.asarray(inputs["scale"], dtype=np.float32))
    bias = np.ascontiguousarray(np.asarray(inputs["bias"], dtype=np.float32))

    if "nc" not in _cache:
        _cache["nc"] = _build()
    nc = _cache["nc"]

    in_maps = [
        {
            "x": np.ascontiguousarray(x[c * BP : (c + 1) * BP]),
            "w": w,
            "scale": scale,
            "bias": bias,
        }
        for c in range(NCORES)
    ]
    trace = os.environ.get("KERNEL_TRACE", "0") == "1"
    if "warm" not in _cache:
        # First execution on a cold device runs slow (model load, power
        # state); do an untraced warmup run so timed runs are steady-state.
        run_bass_kernel_spmd(nc, in_maps, core_ids=list(range(NCORES)), trace=False)
        _cache["warm"] = True
    res = run_bass_kernel_spmd(
        nc, in_maps, core_ids=list(range(NCORES)), trace=trace
    )
    last_results = res
    out = np.concatenate([res.results[c]["out"] for c in range(NCORES)], axis=0)
    return out


# revision 31
# speedup vs baseline: 1.0078x; 1.0078x over previous
"""Trainium2 Bass kernel for BioSphericalCKN1D (dense_cnn).

Computes, for x (32, 4096, 128), W (15, 128, 256), scale (1,1,1), bias (256):

    dot[b,t,f]  = sum_{k,c} x[b,t+k,c] * W[k,c,f]          (VALID conv, T_out = 4082)
    win[b,t]    = sum_{k,c} x[b,t+k,c]^2
    out[b,t,f]  = scale * dot / sqrt(win + 1e-7) + bias

Strategy:
  - Data-parallel over batch: 8 cores x 4 batches each; kernel/scale/bias replicated.
  - Per batch on-core:
      * load x in 1MB super-tiles, TensorE-transpose 128x128 tiles to xT [c, t]
      * ACT Square with fused row-sum accum -> per-t sum-of-squares S
      * sliding 15-window sums of S via 2 matmuls with constant band matrices
      * main conv: per 128-t tile, accumulate 15 float32r matmuls in PSUM
        (stationary = xT slice [c,128t], moving = W[k] [c,256f]) -> psum [t, f]
      * evacuate with ACT Copy scaled by r = scale/sqrt(win+eps) (per-partition),
        DVE add broadcast bias, batched contiguous DMA store.
  - Batch b+1's load/transpose/norm prologue is interleaved into batch b's
    main-matmul loop so the PE never idles (avoids HAM re-throttle).
"""

import os

import numpy as np

import concourse.bacc as bacc
import concourse.bass as bass
import concourse.mybir as mybir
import concourse.tile as tile
from concourse.bass_utils import run_bass_kernel_spmd

B, L, CIN, F, K = 32, 4096, 128, 256, 15
NCORES = 8
BP = B // NCORES          # batches per core
TOUT = L - K + 1          # 4082
NT = L // 128             # 32 row-tiles per batch
LG = 8                    # 128-row tiles per load super-tile (1 MB DMA)
SG = 4                    # j-tiles per store group (512 KB DMA)
EPS = 1e-7

# float32r = TF32-like PE mode: 1 cycle/row vs 4 for float32 (moving dim >= 256).
MM_DT = (
    mybir.dt.float32r
    if os.environ.get("KERNEL_MM_DT", "float32r") == "float32r"
    else mybir.dt.float32
)

_cache: dict = {}
last_results = None


def _build():
    nc = bacc.Bacc("TRN2", target_bir_lowering=False, debug=False, num_devices=NCORES)

    x_d = nc.dram_tensor("x", (BP, L, CIN), mybir.dt.float32, kind="ExternalInput")
    w_d = nc.dram_tensor("w", (K, CIN, F), mybir.dt.float32, kind="ExternalInput")
    scale_d = nc.dram_tensor("scale", (1, 1, 1), mybir.dt.float32, kind="ExternalInput")
    bias_d = nc.dram_tensor("bias", (F,), mybir.dt.float32, kind="ExternalInput")
    out_d = nc.dram_tensor("out", (BP, TOUT, F), mybir.dt.float32, kind="ExternalOutput")

    # Band matrices for the sliding-window sum: win[i*128+p] = sum_k sq[i*128+p+k]
    #   = (A.T @ S[:, i]) + (B.T @ S[:, i+1])   with A[q,p]=1 iff 0<=q-p<=K-1,
    #                                                B[q,p]=1 iff p-q>=128-(K-1)
    q = np.arange(128)[:, None]
    p = np.arange(128)[None, :]
    A_np = ((q - p >= 0) & (q - p <= K - 1)).astype(np.float32)
    B_np = (p - q >= 128 - (K - 1)).astype(np.float32)
    A_d = nc.inline_tensor(A_np, "bandA")
    B_d = nc.inline_tensor(B_np, "bandB")
    I_d = nc.inline_tensor(np.eye(128, dtype=np.float32), "ident")

    XT_COLS = L + 128  # main-mm lhsT slices read up to col 4109; zero-pad tail

    with tile.TileContext(nc) as tc:
        with (
            tc.tile_pool(name="const", bufs=1) as cpool,
            tc.tile_pool(name="xn", bufs=3) as xnpool,
            tc.tile_pool(name="sqs", bufs=3) as sqpool,
            tc.tile_pool(name="xt", bufs=2) as xtpool,
            tc.tile_pool(name="sums", bufs=2) as spool,
            tc.tile_pool(name="small", bufs=2) as smpool,
            tc.tile_pool(name="res", bufs=3) as respool,
            tc.tile_pool(name="pso", bufs=6, space=bass.MemorySpace.PSUM) as psopool,
            tc.tile_pool(name="pst", bufs=1, space=bass.MemorySpace.PSUM) as pstpool,
            tc.tile_pool(name="psw", bufs=1, space=bass.MemorySpace.PSUM) as pswpool,
        ):
            A_sb = cpool.tile([128, 128], mybir.dt.float32, tag="A")
            B_sb = cpool.tile([128, 128], mybir.dt.float32, tag="B")
            ident = cpool.tile([128, 128], mybir.dt.float32, tag="I")
            w_sb = cpool.tile([128, K, F], MM_DT, tag="W")
            bias_bc = cpool.tile([128, F], mybir.dt.float32, tag="bias")
            scale_col = cpool.tile([128, 1], mybir.dt.float32, tag="scale")
            eps_col = cpool.tile([128, 1], mybir.dt.float32, tag="eps")
            nc.vector.memset(eps_col[:], EPS)
            zeros128 = cpool.tile([128, 128], mybir.dt.float32, tag="zeros")
            nc.vector.memset(zeros128[:], 0.0)

            # identity/band consts first on the fast HWDGE queue (tiny, and
            # the very first transposes/win matmuls need them); W first on the
            # gpsimd queue with a casting DMA (fp32 -> float32r rounds, which
            # satisfies the verifier's fp32r-producer rule)
            nc.sync.dma_start(ident[:], I_d[:])
            nc.sync.dma_start(A_sb[:], A_d[:])
            nc.sync.dma_start(B_sb[:], B_d[:])
            # W in four chunks so batch-0's first mains track the arrival
            # (SWDGE casts fp32 -> float32r)
            for k0, k1 in ((0, 4), (4, 8), (8, 12), (12, K)):
                nc.gpsimd.dma_start(
                    w_sb[:, k0:k1, :], w_d[k0:k1].transpose([1, 0, 2])
                )
            nc.gpsimd.dma_start(
                bias_bc[:], bias_d[:].unsqueeze(0).partition_broadcast(128).squeeze(1)
            )
            nc.gpsimd.dma_start(
                scale_col[:],
                scale_d[:].flatten().unsqueeze(0).partition_broadcast(128).squeeze(1),
            )

            # Per-batch persistent tiles, created lazily by the prologue.
            xts = {}
            Ss = {}
            Rs = {}

            def load_group(b, i0, n, engine=None):
                """DMA one super-tile (n 128-row tiles, starting at tile i0)."""
                xn = xnpool.tile([128, n, 128], mybir.dt.float32, tag="xn")
                # src: x[b, i0*128 + i*128 + p, c] -> dest (p, i, c)
                src = x_d[b, i0 * 128 : (i0 + n) * 128, :].rearrange(
                    "(i p) c -> p i c", p=128
                )
                (engine or nc.sync).dma_start(xn[:], src)
                return xn

            def prologue_item(b, i, xn_group):
                """Square+transpose 128-row tile i of batch b from its super-tile."""
                if b not in xts:
                    xts[b] = xtpool.tile([128, XT_COLS], MM_DT, tag="xt", name=f"xt{b}")
                    Ss[b] = spool.tile([128, NT + 1], mybir.dt.float32, tag="S", name=f"S{b}")
                    Rs[b] = smpool.tile([128, NT], mybir.dt.float32, tag="R", name=f"R{b}")
                    nc.vector.tensor_copy(xts[b][:, L:XT_COLS], zeros128[:])
                    nc.vector.memset(Ss[b][:, NT : NT + 1], 0.0)
                xt, S = xts[b], Ss[b]
                if xn_group.ndim == 3:
                    xn_slice = xn_group[:, i % LG, :]
                else:
                    xn_slice = xn_group
                sq = sqpool.tile([128, 128], mybir.dt.float32, tag="sq")
                nc.scalar.activation(
                    sq[:],
                    xn_slice,
                    mybir.ActivationFunctionType.Square,
                    accum_out=S[:, i : i + 1],
                )
                pst = pstpool.tile([128, 128], mybir.dt.float32, tag="pst")
                nc.tensor.transpose(pst[:], xn_slice, ident[:])
                nc.vector.tensor_copy(xt[:, i * 128 : (i + 1) * 128], pst[:])

            def win_r(b, c0=0, c1=NT):
                """r = scale/sqrt(win+eps) for S columns [c0, c1) of batch b."""
                S, R = Ss[b], Rs[b]
                n = c1 - c0
                winp = pswpool.tile([128, n], mybir.dt.float32, tag="win", name=f"win{b}_{c0}")
                nc.tensor.matmul(winp[:], A_sb[:], S[:, c0:c1], start=True, stop=False)
                nc.tensor.matmul(
                    winp[:], B_sb[:], S[:, c0 + 1 : c1 + 1], start=False, stop=True
                )
                sd = smpool.tile([128, n], mybir.dt.float32, tag="sd", name=f"sd{b}_{c0}")
                nc.scalar.activation(
                    sd[:], winp[:], mybir.ActivationFunctionType.Sqrt, bias=eps_col[:]
                )
                rin = smpool.tile([128, n], mybir.dt.float32, tag="rin", name=f"rin{b}_{c0}")
                nc.vector.reciprocal(rin[:], sd[:])
                nc.vector.tensor_scalar_mul(R[:, c0:c1], rin[:], scale_col[:])

            def b0_head(b):
                """Batch-0 cold start: first 9 tiles with graduated load groups
                across both HWDGE queues, then r for the first 8 out-tiles."""
                sizes = [1, 1, 2, 4, 4]
                engines = [nc.scalar, nc.sync, nc.scalar, nc.sync, nc.scalar]
                i0 = 0
                for g, n in enumerate(sizes):
                    xn_group = load_group(b, i0, n, engines[g])
                    for i in range(i0, i0 + n):
                        prologue_item(b, i, xn_group[:, i - i0 : i - i0 + 1, :].squeeze(1))
                        if i == 8:
                            win_r(b, 0, 8)
                    i0 += n

            def b0_tail_item(b, i, xn_holder):
                """Items 9..31 of batch 0, in load groups of 4 on alternating
                queues; win chunks as soon as their S columns are complete.
                NOTE: every R slice must be emitted before any main that reads
                it -- Tile tracks deps by trace order only."""
                if i % 4 == 0:
                    eng = nc.sync if (i // 4) % 2 == 0 else nc.scalar
                    xn_holder[0] = load_group(b, i, 4, eng)
                    xn_holder[1] = i
                prologue_item(
                    b, i, xn_holder[0][:, i - xn_holder[1] : i - xn_holder[1] + 1, :].squeeze(1)
                )
                if i in (16, 24):
                    win_r(b, i - 8, i)
                elif i == NT - 1:
                    win_r(b, 24, NT)

            def main_tile(b, j, ob_group):
                """15 accumulated matmuls + evacuation for 128-row out tile j."""
                xt, R = xts[b], Rs[b]
                po = psopool.tile([128, F], mybir.dt.float32, tag="po")
                for k in range(K):
                    nc.tensor.matmul(
                        po[:],
                        xt[:, j * 128 + k : j * 128 + k + 128],
                        w_sb[:, k, :],
                        start=(k == 0),
                        stop=(k == K - 1),
                    )
                ob_slice = ob_group[:, j % SG, :]
                nc.scalar.activation(
                    ob_slice,
                    po[:],
                    mybir.ActivationFunctionType.Copy,
                    scale=R[:, j : j + 1],
                )
                nc.vector.tensor_add(ob_slice, ob_slice, bias_bc[:])

            def store_group(b, g, ob_group):
                """Store SG evacuated j-tiles as one DMA (split if partial)."""
                j0 = g * SG
                t0 = j0 * 128
                full = min(SG, (TOUT - t0 + 127) // 128)
                # dest iterated as (p, jj, f) to match src tile dims
                last_rows = min(128, TOUT - (j0 + full - 1) * 128)
                if last_rows == 128:
                    dst = out_d[b, t0 : t0 + full * 128, :].rearrange(
                        "(jj p) f -> p jj f", p=128
                    )
                    nc.scalar.dma_start(dst, ob_group[:, 0:full, :])
                else:
                    if full > 1:
                        dst = out_d[b, t0 : t0 + (full - 1) * 128, :].rearrange(
                            "(jj p) f -> p jj f", p=128
                        )
                        nc.scalar.dma_start(dst, ob_group[:, 0 : full - 1, :])
                    tl = (j0 + full - 1) * 128
                    nc.scalar.dma_start(
                        out_d[b, tl : tl + last_rows, :],
                        ob_group[:last_rows, full - 1, :],
                    )

            # ---- schedule ----
            b0_head(0)
            xn_holder = [None, 0]
            for b in range(BP):
                nxt = b + 1
                ob_group = None
                xn_cur = None
                for j in range(NT):
                    if b == 0 and j < 20:
                        # finish batch 0's own prologue under its mains;
                        # emitted BEFORE the main so win chunks precede readers
                        b0_tail_item(0, j + 12, xn_holder)
                    if j % SG == 0:
                        ob_group = respool.tile([128, SG, F], mybir.dt.float32, tag="ob")
                    main_tile(b, j, ob_group)
                    if j % SG == SG - 1:
                        store_group(b, j // SG, ob_group)
                    # interleave next batch's prologue into this batch's mains
                    if nxt < BP:
                        if j % LG == 0:
                            xn_cur = load_group(nxt, j, LG)
                        prologue_item(nxt, j, xn_cur)
                        if j == NT - 1:
                            win_r(nxt)

    nc.compile()
    return nc


def kernel(**inputs: np.ndarray) -> np.ndarray:
    global last_results
    x = np.ascontiguousarray(np.asarray(inputs["inputs"], dtype=np.float32))
    w = np.ascontiguousarray(np.asarray(inputs["kernel"], dtype=np.float32))
    scale = np.ascontiguousarray(np.asarray(inputs["scale"], dtype=np.float32))
    bias = np.ascontiguousarray(np.asarray(inputs["bias"], dtype=np.float32))

    if "nc" not in _cache:
        _cache["nc"] = _build()
    nc = _cache["nc"]

    in_maps = [
        {
            "x": np.ascontiguousarray(x[c * BP : (c + 1) * BP]),
            "w": w,
            "scale": scale,
            "bias": bias,
        }
        for c in range(NCORES)
    ]
    trace = os.environ.get("KERNEL_TRACE", "0") == "1"
    if "warm" not in _cache:
        # First execution on a cold device runs slow (model load, power
        # state); do an untraced warmup run so timed runs are steady-state.
        run_bass_kernel_spmd(nc, in_maps, core_ids=list(range(NCORES)), trace=False)
        _cache["warm"] = True
    res = run_bass_kernel_spmd(
        nc, in_maps, core_ids=list(range(NCORES)), trace=trace
    )
    last_results = res
    out = np.concatenate([res.results[c]["out"] for c in range(NCORES)], axis=0)
    return out


# revision 32
# speedup vs baseline: 1.0085x; 1.0007x over previous
"""Trainium2 Bass kernel for BioSphericalCKN1D (dense_cnn).

Computes, for x (32, 4096, 128), W (15, 128, 256), scale (1,1,1), bias (256):

    dot[b,t,f]  = sum_{k,c} x[b,t+k,c] * W[k,c,f]          (VALID conv, T_out = 4082)
    win[b,t]    = sum_{k,c} x[b,t+k,c]^2
    out[b,t,f]  = scale * dot / sqrt(win + 1e-7) + bias

Strategy:
  - Data-parallel over batch: 8 cores x 4 batches each; kernel/scale/bias replicated.
  - Per batch on-core:
      * load x in 1MB super-tiles, TensorE-transpose 128x128 tiles to xT [c, t]
      * ACT Square with fused row-sum accum -> per-t sum-of-squares S
      * sliding 15-window sums of S via 2 matmuls with constant band matrices
      * main conv: per 128-t tile, accumulate 15 float32r matmuls in PSUM
        (stationary = xT slice [c,128t], moving = W[k] [c,256f]) -> psum [t, f]
      * evacuate with ACT Copy scaled by r = scale/sqrt(win+eps) (per-partition),
        DVE add broadcast bias, batched contiguous DMA store.
  - Batch b+1's load/transpose/norm prologue is interleaved into batch b's
    main-matmul loop so the PE never idles (avoids HAM re-throttle).
"""

import os

import numpy as np

import concourse.bacc as bacc
import concourse.bass as bass
import concourse.mybir as mybir
import concourse.tile as tile
from concourse.bass_utils import run_bass_kernel_spmd

B, L, CIN, F, K = 32, 4096, 128, 256, 15
NCORES = 8
BP = B // NCORES          # batches per core
TOUT = L - K + 1          # 4082
NT = L // 128             # 32 row-tiles per batch
LG = 8                    # 128-row tiles per load super-tile (1 MB DMA)
SG = 4                    # j-tiles per store group (512 KB DMA)
EPS = 1e-7

# float32r = TF32-like PE mode: 1 cycle/row vs 4 for float32 (moving dim >= 256).
MM_DT = (
    mybir.dt.float32r
    if os.environ.get("KERNEL_MM_DT", "float32r") == "float32r"
    else mybir.dt.float32
)

_cache: dict = {}
last_results = None


def _build():
    nc = bacc.Bacc("TRN2", target_bir_lowering=False, debug=False, num_devices=NCORES)

    x_d = nc.dram_tensor("x", (BP, L, CIN), mybir.dt.float32, kind="ExternalInput")
    w_d = nc.dram_tensor("w", (K, CIN, F), mybir.dt.float32, kind="ExternalInput")
    scale_d = nc.dram_tensor("scale", (1, 1, 1), mybir.dt.float32, kind="ExternalInput")
    bias_d = nc.dram_tensor("bias", (F,), mybir.dt.float32, kind="ExternalInput")
    out_d = nc.dram_tensor("out", (BP, TOUT, F), mybir.dt.float32, kind="ExternalOutput")

    # Band matrices for the sliding-window sum: win[i*128+p] = sum_k sq[i*128+p+k]
    #   = (A.T @ S[:, i]) + (B.T @ S[:, i+1])   with A[q,p]=1 iff 0<=q-p<=K-1,
    #                                                B[q,p]=1 iff p-q>=128-(K-1)
    q = np.arange(128)[:, None]
    p = np.arange(128)[None, :]
    A_np = ((q - p >= 0) & (q - p <= K - 1)).astype(np.float32)
    B_np = (p - q >= 128 - (K - 1)).astype(np.float32)
    A_d = nc.inline_tensor(A_np, "bandA")
    B_d = nc.inline_tensor(B_np, "bandB")
    I_d = nc.inline_tensor(np.eye(128, dtype=np.float32), "ident")

    XT_COLS = L + 128  # main-mm lhsT slices read up to col 4109; zero-pad tail

    with tile.TileContext(nc) as tc:
        with (
            tc.tile_pool(name="const", bufs=1) as cpool,
            tc.tile_pool(name="xn", bufs=3) as xnpool,
            tc.tile_pool(name="sqs", bufs=3) as sqpool,
            tc.tile_pool(name="xt", bufs=2) as xtpool,
            tc.tile_pool(name="sums", bufs=2) as spool,
            tc.tile_pool(name="small", bufs=2) as smpool,
            tc.tile_pool(name="res", bufs=3) as respool,
            tc.tile_pool(name="pso", bufs=6, space=bass.MemorySpace.PSUM) as psopool,
            tc.tile_pool(name="pst", bufs=1, space=bass.MemorySpace.PSUM) as pstpool,
            tc.tile_pool(name="psw", bufs=1, space=bass.MemorySpace.PSUM) as pswpool,
        ):
            A_sb = cpool.tile([128, 128], mybir.dt.float32, tag="A")
            B_sb = cpool.tile([128, 128], mybir.dt.float32, tag="B")
            ident = cpool.tile([128, 128], mybir.dt.float32, tag="I")
            w_sb = cpool.tile([128, K, F], MM_DT, tag="W")
            bias_bc = cpool.tile([128, F], mybir.dt.float32, tag="bias")
            scale_col = cpool.tile([128, 1], mybir.dt.float32, tag="scale")
            eps_col = cpool.tile([128, 1], mybir.dt.float32, tag="eps")
            nc.vector.memset(eps_col[:], EPS)
            zeros128 = cpool.tile([128, 128], mybir.dt.float32, tag="zeros")
            nc.vector.memset(zeros128[:], 0.0)

            # identity/band consts first on the fast HWDGE queue (tiny, and
            # the very first transposes/win matmuls need them); W first on the
            # gpsimd queue with a casting DMA (fp32 -> float32r rounds, which
            # satisfies the verifier's fp32r-producer rule)
            nc.sync.dma_start(ident[:], I_d[:])
            nc.sync.dma_start(A_sb[:], A_d[:])
            nc.sync.dma_start(B_sb[:], B_d[:])
            # W in four chunks so batch-0's first mains track the arrival
            # (SWDGE casts fp32 -> float32r)
            for k0, k1 in ((0, 4), (4, 8), (8, 12), (12, K)):
                nc.gpsimd.dma_start(
                    w_sb[:, k0:k1, :], w_d[k0:k1].transpose([1, 0, 2])
                )
            nc.gpsimd.dma_start(
                bias_bc[:], bias_d[:].unsqueeze(0).partition_broadcast(128).squeeze(1)
            )
            nc.gpsimd.dma_start(
                scale_col[:],
                scale_d[:].flatten().unsqueeze(0).partition_broadcast(128).squeeze(1),
            )

            # Per-batch persistent tiles, created lazily by the prologue.
            xts = {}
            Ss = {}
            Rs = {}

            def load_group(b, i0, n, engine=None):
                """DMA one super-tile (n 128-row tiles, starting at tile i0)."""
                xn = xnpool.tile([128, n, 128], mybir.dt.float32, tag="xn")
                # src: x[b, i0*128 + i*128 + p, c] -> dest (p, i, c)
                src = x_d[b, i0 * 128 : (i0 + n) * 128, :].rearrange(
                    "(i p) c -> p i c", p=128
                )
                (engine or nc.sync).dma_start(xn[:], src)
                return xn

            def prologue_item(b, i, xn_group):
                """Square+transpose 128-row tile i of batch b from its super-tile."""
                if b not in xts:
                    xts[b] = xtpool.tile([128, XT_COLS], MM_DT, tag="xt", name=f"xt{b}")
                    Ss[b] = spool.tile([128, NT + 1], mybir.dt.float32, tag="S", name=f"S{b}")
                    Rs[b] = smpool.tile([128, NT], mybir.dt.float32, tag="R", name=f"R{b}")
                    nc.vector.tensor_copy(xts[b][:, L:XT_COLS], zeros128[:])
                    nc.vector.memset(Ss[b][:, NT : NT + 1], 0.0)
                xt, S = xts[b], Ss[b]
                if xn_group.ndim == 3:
                    xn_slice = xn_group[:, i % LG, :]
                else:
                    xn_slice = xn_group
                sq = sqpool.tile([128, 128], mybir.dt.float32, tag="sq")
                nc.scalar.activation(
                    sq[:],
                    xn_slice,
                    mybir.ActivationFunctionType.Square,
                    accum_out=S[:, i : i + 1],
                )
                pst = pstpool.tile([128, 128], mybir.dt.float32, tag="pst")
                nc.tensor.transpose(pst[:], xn_slice, ident[:])
                nc.vector.tensor_copy(xt[:, i * 128 : (i + 1) * 128], pst[:])

            def win_r(b, c0=0, c1=NT):
                """r = scale/sqrt(win+eps) for S columns [c0, c1) of batch b."""
                S, R = Ss[b], Rs[b]
                n = c1 - c0
                winp = pswpool.tile([128, n], mybir.dt.float32, tag="win", name=f"win{b}_{c0}")
                nc.tensor.matmul(winp[:], A_sb[:], S[:, c0:c1], start=True, stop=False)
                nc.tensor.matmul(
                    winp[:], B_sb[:], S[:, c0 + 1 : c1 + 1], start=False, stop=True
                )
                sd = smpool.tile([128, n], mybir.dt.float32, tag="sd", name=f"sd{b}_{c0}")
                nc.scalar.activation(
                    sd[:], winp[:], mybir.ActivationFunctionType.Sqrt, bias=eps_col[:]
                )
                rin = smpool.tile([128, n], mybir.dt.float32, tag="rin", name=f"rin{b}_{c0}")
                nc.vector.reciprocal(rin[:], sd[:])
                nc.vector.tensor_scalar_mul(R[:, c0:c1], rin[:], scale_col[:])

            def b0_head(b):
                """Batch-0 cold start: first 12 tiles with graduated load groups
                across both HWDGE queues, then r for the first 8 out-tiles."""
                sizes = [1, 1, 2, 4, 4]
                engines = [nc.scalar, nc.sync, nc.scalar, nc.sync, nc.scalar]
                i0 = 0
                for g, n in enumerate(sizes):
                    xn_group = load_group(b, i0, n, engines[g])
                    for i in range(i0, i0 + n):
                        prologue_item(b, i, xn_group[:, i - i0 : i - i0 + 1, :].squeeze(1))
                        if i == 8:
                            win_r(b, 0, 8)
                    i0 += n

            def b0_tail_item(b, i, xn_holder):
                """Items 9..31 of batch 0, in load groups of 4 on alternating
                queues; win chunks as soon as their S columns are complete.
                NOTE: every R slice must be emitted before any main that reads
                it -- Tile tracks deps by trace order only."""
                if i % 4 == 0:
                    eng = nc.sync if (i // 4) % 2 == 0 else nc.scalar
                    xn_holder[0] = load_group(b, i, 4, eng)
                    xn_holder[1] = i
                prologue_item(
                    b, i, xn_holder[0][:, i - xn_holder[1] : i - xn_holder[1] + 1, :].squeeze(1)
                )
                if i in (16, 24):
                    win_r(b, i - 8, i)
                elif i == NT - 1:
                    win_r(b, 24, NT)

            def main_tile(b, j, ob_group):
                """15 accumulated matmuls + evacuation for 128-row out tile j."""
                xt, R = xts[b], Rs[b]
                po = psopool.tile([128, F], mybir.dt.float32, tag="po")
                for k in range(K):
                    nc.tensor.matmul(
                        po[:],
                        xt[:, j * 128 + k : j * 128 + k + 128],
                        w_sb[:, k, :],
                        start=(k == 0),
                        stop=(k == K - 1),
                    )
                ob_slice = ob_group[:, j % SG, :]
                nc.scalar.activation(
                    ob_slice,
                    po[:],
                    mybir.ActivationFunctionType.Copy,
                    scale=R[:, j : j + 1],
                )
                nc.vector.tensor_add(ob_slice, ob_slice, bias_bc[:])

            def store_group(b, g, ob_group):
                """Store SG evacuated j-tiles as one DMA (split if partial)."""
                j0 = g * SG
                t0 = j0 * 128
                full = min(SG, (TOUT - t0 + 127) // 128)
                # dest iterated as (p, jj, f) to match src tile dims
                last_rows = min(128, TOUT - (j0 + full - 1) * 128)
                if last_rows == 128:
                    dst = out_d[b, t0 : t0 + full * 128, :].rearrange(
                        "(jj p) f -> p jj f", p=128
                    )
                    nc.scalar.dma_start(dst, ob_group[:, 0:full, :])
                else:
                    if full > 1:
                        dst = out_d[b, t0 : t0 + (full - 1) * 128, :].rearrange(
                            "(jj p) f -> p jj f", p=128
                        )
                        nc.scalar.dma_start(dst, ob_group[:, 0 : full - 1, :])
                    tl = (j0 + full - 1) * 128
                    nc.scalar.dma_start(
                        out_d[b, tl : tl + last_rows, :],
                        ob_group[:last_rows, full - 1, :],
                    )

            # ---- schedule ----
            b0_head(0)
            xn_holder = [None, 0]
            for b in range(BP):
                nxt = b + 1
                ob_group = None
                xn_cur = None
                for j in range(NT):
                    if b == 0 and j < 20:
                        # finish batch 0's own prologue under its mains;
                        # emitted BEFORE the main so win chunks precede readers
                        b0_tail_item(0, j + 12, xn_holder)
                    if j % SG == 0:
                        ob_group = respool.tile([128, SG, F], mybir.dt.float32, tag="ob")
                    main_tile(b, j, ob_group)
                    if j % SG == SG - 1:
                        store_group(b, j // SG, ob_group)
                    # interleave next batch's prologue into this batch's mains
                    if nxt < BP:
                        if j % LG == 0:
                            xn_cur = load_group(nxt, j, LG)
                        prologue_item(nxt, j, xn_cur)
                        if j == NT - 1:
                            win_r(nxt)

    nc.compile()
    return nc


def kernel(**inputs: np.ndarray) -> np.ndarray:
    global last_results
    x = np.ascontiguousarray(np.asarray(inputs["inputs"], dtype=np.float32))
    w = np.ascontiguousarray(np.asarray(inputs["kernel"], dtype=np.float32))
    scale = np.ascontiguousarray(np.asarray(inputs["scale"], dtype=np.float32))
    bias = np.ascontiguousarray(np.asarray(inputs["bias"], dtype=np.float32))

    if "nc" not in _cache:
        _cache["nc"] = _build()
    nc = _cache["nc"]

    in_maps = [
        {
            "x": np.ascontiguousarray(x[c * BP : (c + 1) * BP]),
            "w": w,
            "scale": scale,
            "bias": bias,
        }
        for c in range(NCORES)
    ]
    trace = os.environ.get("KERNEL_TRACE", "0") == "1"
    if "warm" not in _cache:
        # First execution on a cold device runs slow (model load, power
        # state); do an untraced warmup run so timed runs are steady-state.
        run_bass_kernel_spmd(nc, in_maps, core_ids=list(range(NCORES)), trace=False)
        _cache["warm"] = True
    res = run_bass_kernel_spmd(
        nc, in_maps, core_ids=list(range(NCORES)), trace=trace
    )
    last_results = res
    out = np.concatenate([res.results[c]["out"] for c in range(NCORES)], axis=0)
    return out


# revision 34
# speedup vs baseline: 1.0091x; 1.0006x over previous
"""Trainium2 Bass kernel for BioSphericalCKN1D (dense_cnn).

Computes, for x (32, 4096, 128), W (15, 128, 256), scale (1,1,1), bias (256):

    dot[b,t,f]  = sum_{k,c} x[b,t+k,c] * W[k,c,f]          (VALID conv, T_out = 4082)
    win[b,t]    = sum_{k,c} x[b,t+k,c]^2
    out[b,t,f]  = scale * dot / sqrt(win + 1e-7) + bias

Strategy:
  - Data-parallel over batch: 8 cores x 4 batches each; kernel/scale/bias replicated.
  - Per batch on-core:
      * load x in 1MB super-tiles, TensorE-transpose 128x128 tiles to xT [c, t]
      * ACT Square with fused row-sum accum -> per-t sum-of-squares S
      * sliding 15-window sums of S via 2 matmuls with constant band matrices
      * main conv: per 128-t tile, accumulate 15 float32r matmuls in PSUM
        (stationary = xT slice [c,128t], moving = W[k] [c,256f]) -> psum [t, f]
      * evacuate with ACT Copy scaled by r = scale/sqrt(win+eps) (per-partition),
        DVE add broadcast bias, batched contiguous DMA store.
  - Batch b+1's load/transpose/norm prologue is interleaved into batch b's
    main-matmul loop so the PE never idles (avoids HAM re-throttle).
"""

import os

import numpy as np

import concourse.bacc as bacc
import concourse.bass as bass
import concourse.mybir as mybir
import concourse.tile as tile
from concourse.bass_utils import run_bass_kernel_spmd

B, L, CIN, F, K = 32, 4096, 128, 256, 15
NCORES = 8
BP = B // NCORES          # batches per core
TOUT = L - K + 1          # 4082
NT = L // 128             # 32 row-tiles per batch
LG = 8                    # 128-row tiles per load super-tile (1 MB DMA)
SG = 4                    # j-tiles per store group (512 KB DMA)
EPS = 1e-7

# float32r = TF32-like PE mode: 1 cycle/row vs 4 for float32 (moving dim >= 256).
MM_DT = (
    mybir.dt.float32r
    if os.environ.get("KERNEL_MM_DT", "float32r") == "float32r"
    else mybir.dt.float32
)

_cache: dict = {}
last_results = None


def _build():
    nc = bacc.Bacc("TRN2", target_bir_lowering=False, debug=False, num_devices=NCORES)

    x_d = nc.dram_tensor("x", (BP, L, CIN), mybir.dt.float32, kind="ExternalInput")
    w_d = nc.dram_tensor("w", (K, CIN, F), mybir.dt.float32, kind="ExternalInput")
    scale_d = nc.dram_tensor("scale", (1, 1, 1), mybir.dt.float32, kind="ExternalInput")
    bias_d = nc.dram_tensor("bias", (F,), mybir.dt.float32, kind="ExternalInput")
    out_d = nc.dram_tensor("out", (BP, TOUT, F), mybir.dt.float32, kind="ExternalOutput")

    # Band matrices for the sliding-window sum: win[i*128+p] = sum_k sq[i*128+p+k]
    #   = (A.T @ S[:, i]) + (B.T @ S[:, i+1])   with A[q,p]=1 iff 0<=q-p<=K-1,
    #                                                B[q,p]=1 iff p-q>=128-(K-1)
    q = np.arange(128)[:, None]
    p = np.arange(128)[None, :]
    A_np = ((q - p >= 0) & (q - p <= K - 1)).astype(np.float32)
    B_np = (p - q >= 128 - (K - 1)).astype(np.float32)
    A_d = nc.inline_tensor(A_np, "bandA")
    B_d = nc.inline_tensor(B_np, "bandB")
    I_d = nc.inline_tensor(np.eye(128, dtype=np.float32), "ident")

    XT_COLS = L + 128  # main-mm lhsT slices read up to col 4109; zero-pad tail

    with tile.TileContext(nc) as tc:
        with (
            tc.tile_pool(name="const", bufs=1) as cpool,
            tc.tile_pool(name="xn", bufs=3) as xnpool,
            tc.tile_pool(name="sqs", bufs=3) as sqpool,
            tc.tile_pool(name="xt", bufs=2) as xtpool,
            tc.tile_pool(name="sums", bufs=2) as spool,
            tc.tile_pool(name="small", bufs=2) as smpool,
            tc.tile_pool(name="res", bufs=3) as respool,
            tc.tile_pool(name="pso", bufs=6, space=bass.MemorySpace.PSUM) as psopool,
            tc.tile_pool(name="pst", bufs=1, space=bass.MemorySpace.PSUM) as pstpool,
            tc.tile_pool(name="psw", bufs=1, space=bass.MemorySpace.PSUM) as pswpool,
        ):
            A_sb = cpool.tile([128, 128], mybir.dt.float32, tag="A")
            B_sb = cpool.tile([128, 128], mybir.dt.float32, tag="B")
            ident = cpool.tile([128, 128], mybir.dt.float32, tag="I")
            w_sb = cpool.tile([128, K, F], MM_DT, tag="W")
            bias_bc = cpool.tile([128, F], mybir.dt.float32, tag="bias")
            scale_col = cpool.tile([128, 1], mybir.dt.float32, tag="scale")
            eps_col = cpool.tile([128, 1], mybir.dt.float32, tag="eps")
            nc.vector.memset(eps_col[:], EPS)
            zeros128 = cpool.tile([128, 128], mybir.dt.float32, tag="zeros")
            nc.vector.memset(zeros128[:], 0.0)

            # identity/band consts first on the fast HWDGE queue (tiny, and
            # the very first transposes/win matmuls need them); W first on the
            # gpsimd queue with a casting DMA (fp32 -> float32r rounds, which
            # satisfies the verifier's fp32r-producer rule)
            nc.sync.dma_start(ident[:], I_d[:])
            nc.sync.dma_start(A_sb[:], A_d[:])
            nc.sync.dma_start(B_sb[:], B_d[:])
            # W in four chunks so batch-0's first mains track the arrival
            # (SWDGE casts fp32 -> float32r)
            for k0, k1 in ((0, 4), (4, 8), (8, 12), (12, K)):
                nc.gpsimd.dma_start(
                    w_sb[:, k0:k1, :], w_d[k0:k1].transpose([1, 0, 2])
                )
            nc.gpsimd.dma_start(
                bias_bc[:], bias_d[:].unsqueeze(0).partition_broadcast(128).squeeze(1)
            )
            nc.gpsimd.dma_start(
                scale_col[:],
                scale_d[:].flatten().unsqueeze(0).partition_broadcast(128).squeeze(1),
            )

            # Per-batch persistent tiles, created lazily by the prologue.
            xts = {}
            Ss = {}
            Rs = {}

            def load_group(b, i0, n, engine=None):
                """DMA one super-tile (n 128-row tiles, starting at tile i0)."""
                xn = xnpool.tile([128, n, 128], mybir.dt.float32, tag="xn")
                # src: x[b, i0*128 + i*128 + p, c] -> dest (p, i, c)
                src = x_d[b, i0 * 128 : (i0 + n) * 128, :].rearrange(
                    "(i p) c -> p i c", p=128
                )
                (engine or nc.sync).dma_start(xn[:], src)
                return xn

            def prologue_item(b, i, xn_group):
                """Square+transpose 128-row tile i of batch b from its super-tile."""
                if b not in xts:
                    xts[b] = xtpool.tile([128, XT_COLS], MM_DT, tag="xt", name=f"xt{b}")
                    Ss[b] = spool.tile([128, NT + 1], mybir.dt.float32, tag="S", name=f"S{b}")
                    Rs[b] = smpool.tile([128, NT], mybir.dt.float32, tag="R", name=f"R{b}")
                    nc.vector.tensor_copy(xts[b][:, L:XT_COLS], zeros128[:])
                    nc.vector.memset(Ss[b][:, NT : NT + 1], 0.0)
                xt, S = xts[b], Ss[b]
                if xn_group.ndim == 3:
                    xn_slice = xn_group[:, i % LG, :]
                else:
                    xn_slice = xn_group
                sq = sqpool.tile([128, 128], mybir.dt.float32, tag="sq")
                nc.scalar.activation(
                    sq[:],
                    xn_slice,
                    mybir.ActivationFunctionType.Square,
                    accum_out=S[:, i : i + 1],
                )
                pst = pstpool.tile([128, 128], mybir.dt.float32, tag="pst")
                nc.tensor.transpose(pst[:], xn_slice, ident[:])
                nc.vector.tensor_copy(xt[:, i * 128 : (i + 1) * 128], pst[:])

            def win_r(b, c0=0, c1=NT):
                """r = scale/sqrt(win+eps) for S columns [c0, c1) of batch b."""
                S, R = Ss[b], Rs[b]
                n = c1 - c0
                winp = pswpool.tile([128, n], mybir.dt.float32, tag="win", name=f"win{b}_{c0}")
                nc.tensor.matmul(winp[:], A_sb[:], S[:, c0:c1], start=True, stop=False)
                nc.tensor.matmul(
                    winp[:], B_sb[:], S[:, c0 + 1 : c1 + 1], start=False, stop=True
                )
                sd = smpool.tile([128, n], mybir.dt.float32, tag="sd", name=f"sd{b}_{c0}")
                nc.scalar.activation(
                    sd[:], winp[:], mybir.ActivationFunctionType.Sqrt, bias=eps_col[:]
                )
                rin = smpool.tile([128, n], mybir.dt.float32, tag="rin", name=f"rin{b}_{c0}")
                nc.vector.reciprocal(rin[:], sd[:])
                nc.vector.tensor_scalar_mul(R[:, c0:c1], rin[:], scale_col[:])

            def b0_head(b):
                """Batch-0 cold start: first 9 tiles with graduated load groups
                across both HWDGE queues, then r for the first 8 out-tiles."""
                sizes = [1, 1, 2, 4, 4]
                engines = [nc.scalar, nc.sync, nc.scalar, nc.sync, nc.scalar]
                i0 = 0
                for g, n in enumerate(sizes):
                    xn_group = load_group(b, i0, n, engines[g])
                    for i in range(i0, i0 + n):
                        prologue_item(b, i, xn_group[:, i - i0 : i - i0 + 1, :].squeeze(1))
                        if i == 8:
                            win_r(b, 0, 8)
                    i0 += n

            def b0_tail_item(b, i, xn_holder):
                """Items 9..31 of batch 0, in load groups of 4 on alternating
                queues; win chunks as soon as their S columns are complete.
                NOTE: every R slice must be emitted before any main that reads
                it -- Tile tracks deps by trace order only."""
                if i % 4 == 0:
                    eng = nc.sync if (i // 4) % 2 == 0 else nc.scalar
                    xn_holder[0] = load_group(b, i, 4, eng)
                    xn_holder[1] = i
                prologue_item(
                    b, i, xn_holder[0][:, i - xn_holder[1] : i - xn_holder[1] + 1, :].squeeze(1)
                )
                if i in (16, 24):
                    win_r(b, i - 8, i)
                elif i == NT - 1:
                    win_r(b, 24, NT)

            def main_tile(b, j, ob_group):
                """15 accumulated matmuls + evacuation for 128-row out tile j."""
                xt, R = xts[b], Rs[b]
                po = psopool.tile([128, F], mybir.dt.float32, tag="po")
                for k in range(K):
                    nc.tensor.matmul(
                        po[:],
                        xt[:, j * 128 + k : j * 128 + k + 128],
                        w_sb[:, k, :],
                        start=(k == 0),
                        stop=(k == K - 1),
                    )
                ob_slice = ob_group[:, j % SG, :]
                nc.scalar.activation(
                    ob_slice,
                    po[:],
                    mybir.ActivationFunctionType.Copy,
                    scale=R[:, j : j + 1],
                )
                nc.vector.tensor_add(ob_slice, ob_slice, bias_bc[:])

            def store_group(b, g, ob_group):
                """Store SG evacuated j-tiles as one DMA (split if partial)."""
                j0 = g * SG
                t0 = j0 * 128
                full = min(SG, (TOUT - t0 + 127) // 128)
                # dest iterated as (p, jj, f) to match src tile dims
                last_rows = min(128, TOUT - (j0 + full - 1) * 128)
                if last_rows == 128:
                    dst = out_d[b, t0 : t0 + full * 128, :].rearrange(
                        "(jj p) f -> p jj f", p=128
                    )
                    nc.scalar.dma_start(dst, ob_group[:, 0:full, :])
                else:
                    if full > 1:
                        dst = out_d[b, t0 : t0 + (full - 1) * 128, :].rearrange(
                            "(jj p) f -> p jj f", p=128
                        )
                        nc.scalar.dma_start(dst, ob_group[:, 0 : full - 1, :])
                    tl = (j0 + full - 1) * 128
                    nc.scalar.dma_start(
                        out_d[b, tl : tl + last_rows, :],
                        ob_group[:last_rows, full - 1, :],
                    )

            # ---- schedule ----
            b0_head(0)
            xn_holder = [None, 0]
            for b in range(BP):
                nxt = b + 1
                ob_group = None
                xn_cur = None
                for j in range(NT):
                    if b == 0 and j < 20:
                        # finish batch 0's own prologue under its mains;
                        # emitted BEFORE the main so win chunks precede readers
                        b0_tail_item(0, j + 12, xn_holder)
                    if j % SG == 0:
                        ob_group = respool.tile([128, SG, F], mybir.dt.float32, tag="ob")
                    main_tile(b, j, ob_group)
                    if j % SG == SG - 1:
                        store_group(b, j // SG, ob_group)
                    # interleave next batch's prologue into this batch's mains
                    if nxt < BP:
                        if j % LG == 0:
                            xn_cur = load_group(nxt, j, LG)
                        prologue_item(nxt, j, xn_cur)
                        if j == NT - 1:
                            win_r(nxt)

    nc.compile()
    return nc


def kernel(**inputs: np.ndarray) -> np.ndarray:
    global last_results
    x = np.ascontiguousarray(np.asarray(inputs["inputs"], dtype=np.float32))
    w = np.ascontiguousarray(np.asarray(inputs["kernel"], dtype=np.float32))
    scale = np.ascontiguousarray(np.asarray(inputs["scale"], dtype=np.float32))
    bias = np.ascontiguousarray(np.asarray(inputs["bias"], dtype=np.float32))

    if "nc" not in _cache:
        _cache["nc"] = _build()
    nc = _cache["nc"]

    in_maps = [
        {
            "x": np.ascontiguousarray(x[c * BP : (c + 1) * BP]),
            "w": w,
            "scale": scale,
            "bias": bias,
        }
        for c in range(NCORES)
    ]
    trace = os.environ.get("KERNEL_TRACE", "0") == "1"
    if "warm" not in _cache:
        # First execution on a cold device runs slow (model load, power
        # state); do an untraced warmup run so timed runs are steady-state.
        run_bass_kernel_spmd(nc, in_maps, core_ids=list(range(NCORES)), trace=False)
        _cache["warm"] = True
    res = run_bass_kernel_spmd(
        nc, in_maps, core_ids=list(range(NCORES)), trace=trace
    )
    last_results = res
    out = np.concatenate([res.results[c]["out"] for c in range(NCORES)], axis=0)
    return out


# revision 35
# speedup vs baseline: 1.0154x; 1.0063x over previous
"""Trainium2 Bass kernel for BioSphericalCKN1D (dense_cnn).

Computes, for x (32, 4096, 128), W (15, 128, 256), scale (1,1,1), bias (256):

    dot[b,t,f]  = sum_{k,c} x[b,t+k,c] * W[k,c,f]          (VALID conv, T_out = 4082)
    win[b,t]    = sum_{k,c} x[b,t+k,c]^2
    out[b,t,f]  = scale * dot / sqrt(win + 1e-7) + bias

Strategy:
  - Data-parallel over batch: 8 cores x 4 batches each; kernel/scale/bias replicated.
  - Per batch on-core:
      * load x in 1MB super-tiles, TensorE-transpose 128x128 tiles to xT [c, t]
      * ACT Square with fused row-sum accum -> per-t sum-of-squares S
      * sliding 15-window sums of S via 2 matmuls with constant band matrices
      * main conv: per 128-t tile, accumulate 15 float32r matmuls in PSUM
        (stationary = xT slice [c,128t], moving = W[k] [c,256f]) -> psum [t, f]
      * evacuate with ACT Copy scaled by r = scale/sqrt(win+eps) (per-partition),
        DVE add broadcast bias, batched contiguous DMA store.
  - Batch b+1's load/transpose/norm prologue is interleaved into batch b's
    main-matmul loop so the PE never idles (avoids HAM re-throttle).
"""

import os

import numpy as np

import concourse.bacc as bacc
import concourse.bass as bass
import concourse.mybir as mybir
import concourse.tile as tile
from concourse.bass_utils import run_bass_kernel_spmd

B, L, CIN, F, K = 32, 4096, 128, 256, 15
NCORES = 8
BP = B // NCORES          # batches per core
TOUT = L - K + 1          # 4082
NT = L // 128             # 32 row-tiles per batch
LG = 8                    # 128-row tiles per load super-tile (1 MB DMA)
SG = 4                    # j-tiles per store group (512 KB DMA)
EPS = 1e-7

# float32r = TF32-like PE mode: 1 cycle/row vs 4 for float32 (moving dim >= 256).
MM_DT = (
    mybir.dt.float32r
    if os.environ.get("KERNEL_MM_DT", "float32r") == "float32r"
    else mybir.dt.float32
)

_cache: dict = {}
last_results = None


def _build():
    nc = bacc.Bacc("TRN2", target_bir_lowering=False, debug=False, num_devices=NCORES)

    x_d = nc.dram_tensor("x", (BP, L, CIN), mybir.dt.float32, kind="ExternalInput")
    w_d = nc.dram_tensor("w", (K, CIN, F), mybir.dt.float32, kind="ExternalInput")
    scale_d = nc.dram_tensor("scale", (1, 1, 1), mybir.dt.float32, kind="ExternalInput")
    bias_d = nc.dram_tensor("bias", (F,), mybir.dt.float32, kind="ExternalInput")
    out_d = nc.dram_tensor("out", (BP, TOUT, F), mybir.dt.float32, kind="ExternalOutput")

    # Band matrices for the sliding-window sum: win[i*128+p] = sum_k sq[i*128+p+k]
    #   = (A.T @ S[:, i]) + (B.T @ S[:, i+1])   with A[q,p]=1 iff 0<=q-p<=K-1,
    #                                                B[q,p]=1 iff p-q>=128-(K-1)
    q = np.arange(128)[:, None]
    p = np.arange(128)[None, :]
    A_np = ((q - p >= 0) & (q - p <= K - 1)).astype(np.float32)
    B_np = (p - q >= 128 - (K - 1)).astype(np.float32)
    A_d = nc.inline_tensor(A_np, "bandA")
    B_d = nc.inline_tensor(B_np, "bandB")
    I_d = nc.inline_tensor(np.eye(128, dtype=np.float32), "ident")

    XT_COLS = L + 128  # main-mm lhsT slices read up to col 4109; zero-pad tail

    with tile.TileContext(nc) as tc:
        with (
            tc.tile_pool(name="const", bufs=1) as cpool,
            tc.tile_pool(name="xn", bufs=4) as xnpool,
            tc.tile_pool(name="sqs", bufs=3) as sqpool,
            tc.tile_pool(name="xt", bufs=2) as xtpool,
            tc.tile_pool(name="sums", bufs=2) as spool,
            tc.tile_pool(name="small", bufs=2) as smpool,
            tc.tile_pool(name="res", bufs=4) as respool,
            tc.tile_pool(name="pso", bufs=6, space=bass.MemorySpace.PSUM) as psopool,
            tc.tile_pool(name="pst", bufs=1, space=bass.MemorySpace.PSUM) as pstpool,
            tc.tile_pool(name="psw", bufs=1, space=bass.MemorySpace.PSUM) as pswpool,
        ):
            A_sb = cpool.tile([128, 128], mybir.dt.float32, tag="A")
            B_sb = cpool.tile([128, 128], mybir.dt.float32, tag="B")
            ident = cpool.tile([128, 128], mybir.dt.float32, tag="I")
            w_sb = cpool.tile([128, K, F], MM_DT, tag="W")
            bias_bc = cpool.tile([128, F], mybir.dt.float32, tag="bias")
            scale_col = cpool.tile([128, 1], mybir.dt.float32, tag="scale")
            eps_col = cpool.tile([128, 1], mybir.dt.float32, tag="eps")
            nc.vector.memset(eps_col[:], EPS)
            zeros128 = cpool.tile([128, 128], mybir.dt.float32, tag="zeros")
            nc.vector.memset(zeros128[:], 0.0)

            # identity/band consts first on the fast HWDGE queue (tiny, and
            # the very first transposes/win matmuls need them); W first on the
            # gpsimd queue with a casting DMA (fp32 -> float32r rounds, which
            # satisfies the verifier's fp32r-producer rule)
            nc.sync.dma_start(ident[:], I_d[:])
            nc.sync.dma_start(A_sb[:], A_d[:])
            nc.sync.dma_start(B_sb[:], B_d[:])
            # W in four chunks so batch-0's first mains track the arrival
            # (SWDGE casts fp32 -> float32r)
            for k0, k1 in ((0, 2), (2, 4), (4, 7), (7, 11), (11, K)):
                nc.gpsimd.dma_start(
                    w_sb[:, k0:k1, :], w_d[k0:k1].transpose([1, 0, 2])
                )
            nc.gpsimd.dma_start(
                bias_bc[:], bias_d[:].unsqueeze(0).partition_broadcast(128).squeeze(1)
            )
            nc.gpsimd.dma_start(
                scale_col[:],
                scale_d[:].flatten().unsqueeze(0).partition_broadcast(128).squeeze(1),
            )

            # Per-batch persistent tiles, created lazily by the prologue.
            xts = {}
            Ss = {}
            Rs = {}

            def load_group(b, i0, n, engine=None):
                """DMA one super-tile (n 128-row tiles, starting at tile i0)."""
                xn = xnpool.tile([128, n, 128], mybir.dt.float32, tag="xn")
                # src: x[b, i0*128 + i*128 + p, c] -> dest (p, i, c)
                src = x_d[b, i0 * 128 : (i0 + n) * 128, :].rearrange(
                    "(i p) c -> p i c", p=128
                )
                (engine or nc.sync).dma_start(xn[:], src)
                return xn

            def prologue_item(b, i, xn_group):
                """Square+transpose 128-row tile i of batch b from its super-tile."""
                if b not in xts:
                    xts[b] = xtpool.tile([128, XT_COLS], MM_DT, tag="xt", name=f"xt{b}")
                    Ss[b] = spool.tile([128, NT + 1], mybir.dt.float32, tag="S", name=f"S{b}")
                    Rs[b] = smpool.tile([128, NT], mybir.dt.float32, tag="R", name=f"R{b}")
                    nc.vector.tensor_copy(xts[b][:, L:XT_COLS], zeros128[:])
                    nc.vector.memset(Ss[b][:, NT : NT + 1], 0.0)
                xt, S = xts[b], Ss[b]
                if xn_group.ndim == 3:
                    xn_slice = xn_group[:, i % LG, :]
                else:
                    xn_slice = xn_group
                sq = sqpool.tile([128, 128], mybir.dt.float32, tag="sq")
                nc.scalar.activation(
                    sq[:],
                    xn_slice,
                    mybir.ActivationFunctionType.Square,
                    accum_out=S[:, i : i + 1],
                )
                pst = pstpool.tile([128, 128], mybir.dt.float32, tag="pst")
                nc.tensor.transpose(pst[:], xn_slice, ident[:])
                nc.vector.tensor_copy(xt[:, i * 128 : (i + 1) * 128], pst[:])

            def win_r(b, c0=0, c1=NT):
                """r = scale/sqrt(win+eps) for S columns [c0, c1) of batch b."""
                S, R = Ss[b], Rs[b]
                n = c1 - c0
                winp = pswpool.tile([128, n], mybir.dt.float32, tag="win", name=f"win{b}_{c0}")
                nc.tensor.matmul(winp[:], A_sb[:], S[:, c0:c1], start=True, stop=False)
                nc.tensor.matmul(
                    winp[:], B_sb[:], S[:, c0 + 1 : c1 + 1], start=False, stop=True
                )
                sd = smpool.tile([128, n], mybir.dt.float32, tag="sd", name=f"sd{b}_{c0}")
                nc.scalar.activation(
                    sd[:], winp[:], mybir.ActivationFunctionType.Sqrt, bias=eps_col[:]
                )
                rin = smpool.tile([128, n], mybir.dt.float32, tag="rin", name=f"rin{b}_{c0}")
                nc.vector.reciprocal(rin[:], sd[:])
                nc.vector.tensor_scalar_mul(R[:, c0:c1], rin[:], scale_col[:])

            def b0_head(b):
                """Batch-0 cold start: first 9 tiles with graduated load groups
                across both HWDGE queues, then r for the first 8 out-tiles."""
                sizes = [1, 1, 2, 4, 4]
                engines = [nc.scalar, nc.sync, nc.scalar, nc.sync, nc.scalar]
                i0 = 0
                for g, n in enumerate(sizes):
                    xn_group = load_group(b, i0, n, engines[g])
                    for i in range(i0, i0 + n):
                        prologue_item(b, i, xn_group[:, i - i0 : i - i0 + 1, :].squeeze(1))
                        if i == 8:
                            win_r(b, 0, 8)
                    i0 += n

            def b0_tail_item(b, i, xn_holder):
                """Items 9..31 of batch 0, in load groups of 4 on alternating
                queues; win chunks as soon as their S columns are complete.
                NOTE: every R slice must be emitted before any main that reads
                it -- Tile tracks deps by trace order only."""
                if i % 4 == 0:
                    eng = nc.sync if (i // 4) % 2 == 0 else nc.scalar
                    xn_holder[0] = load_group(b, i, 4, eng)
                    xn_holder[1] = i
                prologue_item(
                    b, i, xn_holder[0][:, i - xn_holder[1] : i - xn_holder[1] + 1, :].squeeze(1)
                )
                if i in (16, 24):
                    win_r(b, i - 8, i)
                elif i == NT - 1:
                    win_r(b, 24, NT)

            def main_tile(b, j, ob_group):
                """15 accumulated matmuls + evacuation for 128-row out tile j."""
                xt, R = xts[b], Rs[b]
                po = psopool.tile([128, F], mybir.dt.float32, tag="po")
                for k in range(K):
                    nc.tensor.matmul(
                        po[:],
                        xt[:, j * 128 + k : j * 128 + k + 128],
                        w_sb[:, k, :],
                        start=(k == 0),
                        stop=(k == K - 1),
                    )
                ob_slice = ob_group[:, j % SG, :]
                nc.scalar.activation(
                    ob_slice,
                    po[:],
                    mybir.ActivationFunctionType.Copy,
                    scale=R[:, j : j + 1],
                )
                nc.vector.tensor_add(ob_slice, ob_slice, bias_bc[:])

            def store_group(b, g, ob_group):
                """Store SG evacuated j-tiles as one DMA (split if partial)."""
                j0 = g * SG
                t0 = j0 * 128
                full = min(SG, (TOUT - t0 + 127) // 128)
                # dest iterated as (p, jj, f) to match src tile dims
                last_rows = min(128, TOUT - (j0 + full - 1) * 128)
                if last_rows == 128:
                    dst = out_d[b, t0 : t0 + full * 128, :].rearrange(
                        "(jj p) f -> p jj f", p=128
                    )
                    nc.scalar.dma_start(dst, ob_group[:, 0:full, :])
                else:
                    if full > 1:
                        dst = out_d[b, t0 : t0 + (full - 1) * 128, :].rearrange(
                            "(jj p) f -> p jj f", p=128
                        )
                        nc.scalar.dma_start(dst, ob_group[:, 0 : full - 1, :])
                    tl = (j0 + full - 1) * 128
                    nc.scalar.dma_start(
                        out_d[b, tl : tl + last_rows, :],
                        ob_group[:last_rows, full - 1, :],
                    )

            # ---- schedule ----
            b0_head(0)
            xn_holder = [None, 0]
            for b in range(BP):
                nxt = b + 1
                ob_group = None
                xn_cur = None
                for j in range(NT):
                    if b == 0 and j < 20:
                        # finish batch 0's own prologue under its mains;
                        # emitted BEFORE the main so win chunks precede readers
                        b0_tail_item(0, j + 12, xn_holder)
                    if j % SG == 0:
                        ob_group = respool.tile([128, SG, F], mybir.dt.float32, tag="ob")
                    main_tile(b, j, ob_group)
                    if j % SG == SG - 1:
                        store_group(b, j // SG, ob_group)
                    # interleave next batch's prologue into this batch's mains
                    if nxt < BP:
                        if j % LG == 0:
                            xn_cur = load_group(nxt, j, LG)
                        prologue_item(nxt, j, xn_cur)
                        if j == NT - 1:
                            win_r(nxt)

    nc.compile()
    return nc


def kernel(**inputs: np.ndarray) -> np.ndarray:
    global last_results
    x = np.ascontiguousarray(np.asarray(inputs["inputs"], dtype=np.float32))
    w = np.ascontiguousarray(np.asarray(inputs["kernel"], dtype=np.float32))
    scale = np.ascontiguousarray(np.asarray(inputs["scale"], dtype=np.float32))
    bias = np.ascontiguousarray(np.asarray(inputs["bias"], dtype=np.float32))

    if "nc" not in _cache:
        _cache["nc"] = _build()
    nc = _cache["nc"]

    in_maps = [
        {
            "x": np.ascontiguousarray(x[c * BP : (c + 1) * BP]),
            "w": w,
            "scale": scale,
            "bias": bias,
        }
        for c in range(NCORES)
    ]
    trace = os.environ.get("KERNEL_TRACE", "0") == "1"
    if "warm" not in _cache:
        # First execution on a cold device runs slow (model load, power
        # state); do an untraced warmup run so timed runs are steady-state.
        run_bass_kernel_spmd(nc, in_maps, core_ids=list(range(NCORES)), trace=False)
        _cache["warm"] = True
    res = run_bass_kernel_spmd(
        nc, in_maps, core_ids=list(range(NCORES)), trace=trace
    )
    last_results = res
    out = np.concatenate([res.results[c]["out"] for c in range(NCORES)], axis=0)
    return out


# revision 37
# speedup vs baseline: 1.0158x; 1.0003x over previous
"""Trainium2 Bass kernel for BioSphericalCKN1D (dense_cnn).

Computes, for x (32, 4096, 128), W (15, 128, 256), scale (1,1,1), bias (256):

    dot[b,t,f]  = sum_{k,c} x[b,t+k,c] * W[k,c,f]          (VALID conv, T_out = 4082)
    win[b,t]    = sum_{k,c} x[b,t+k,c]^2
    out[b,t,f]  = scale * dot / sqrt(win + 1e-7) + bias

Strategy:
  - Data-parallel over batch: 8 cores x 4 batches each; kernel/scale/bias replicated.
  - Per batch on-core:
      * load x in 1MB super-tiles, TensorE-transpose 128x128 tiles to xT [c, t]
      * ACT Square with fused row-sum accum -> per-t sum-of-squares S
      * sliding 15-window sums of S via 2 matmuls with constant band matrices
      * main conv: per 128-t tile, accumulate 15 float32r matmuls in PSUM
        (stationary = xT slice [c,128t], moving = W[k] [c,256f]) -> psum [t, f]
      * evacuate with ACT Copy scaled by r = scale/sqrt(win+eps) (per-partition),
        DVE add broadcast bias, batched contiguous DMA store.
  - Batch b+1's load/transpose/norm prologue is interleaved into batch b's
    main-matmul loop so the PE never idles (avoids HAM re-throttle).
"""

import os

import numpy as np

import concourse.bacc as bacc
import concourse.bass as bass
import concourse.mybir as mybir
import concourse.tile as tile
from concourse.bass_utils import run_bass_kernel_spmd

B, L, CIN, F, K = 32, 4096, 128, 256, 15
NCORES = 8
BP = B // NCORES          # batches per core
TOUT = L - K + 1          # 4082
NT = L // 128             # 32 row-tiles per batch
LG = 8                    # 128-row tiles per load super-tile (1 MB DMA)
SG = 4                    # j-tiles per store group (512 KB DMA)
EPS = 1e-7

# float32r = TF32-like PE mode: 1 cycle/row vs 4 for float32 (moving dim >= 256).
MM_DT = (
    mybir.dt.float32r
    if os.environ.get("KERNEL_MM_DT", "float32r") == "float32r"
    else mybir.dt.float32
)

_cache: dict = {}
last_results = None


def _build():
    nc = bacc.Bacc("TRN2", target_bir_lowering=False, debug=False, num_devices=NCORES)

    x_d = nc.dram_tensor("x", (BP, L, CIN), mybir.dt.float32, kind="ExternalInput")
    w_d = nc.dram_tensor("w", (K, CIN, F), mybir.dt.float32, kind="ExternalInput")
    scale_d = nc.dram_tensor("scale", (1, 1, 1), mybir.dt.float32, kind="ExternalInput")
    bias_d = nc.dram_tensor("bias", (F,), mybir.dt.float32, kind="ExternalInput")
    out_d = nc.dram_tensor("out", (BP, TOUT, F), mybir.dt.float32, kind="ExternalOutput")

    # Band matrices for the sliding-window sum: win[i*128+p] = sum_k sq[i*128+p+k]
    #   = (A.T @ S[:, i]) + (B.T @ S[:, i+1])   with A[q,p]=1 iff 0<=q-p<=K-1,
    #                                                B[q,p]=1 iff p-q>=128-(K-1)
    q = np.arange(128)[:, None]
    p = np.arange(128)[None, :]
    A_np = ((q - p >= 0) & (q - p <= K - 1)).astype(np.float32)
    B_np = (p - q >= 128 - (K - 1)).astype(np.float32)
    A_d = nc.inline_tensor(A_np, "bandA")
    B_d = nc.inline_tensor(B_np, "bandB")
    I_d = nc.inline_tensor(np.eye(128, dtype=np.float32), "ident")

    XT_COLS = L + 128  # main-mm lhsT slices read up to col 4109; zero-pad tail

    with tile.TileContext(nc) as tc:
        with (
            tc.tile_pool(name="const", bufs=1) as cpool,
            tc.tile_pool(name="xn", bufs=4) as xnpool,
            tc.tile_pool(name="sqs", bufs=3) as sqpool,
            tc.tile_pool(name="xt", bufs=2) as xtpool,
            tc.tile_pool(name="sums", bufs=2) as spool,
            tc.tile_pool(name="small", bufs=2) as smpool,
            tc.tile_pool(name="res", bufs=4) as respool,
            tc.tile_pool(name="pso", bufs=6, space=bass.MemorySpace.PSUM) as psopool,
            tc.tile_pool(name="pst", bufs=1, space=bass.MemorySpace.PSUM) as pstpool,
            tc.tile_pool(name="psw", bufs=1, space=bass.MemorySpace.PSUM) as pswpool,
        ):
            A_sb = cpool.tile([128, 128], mybir.dt.float32, tag="A")
            B_sb = cpool.tile([128, 128], mybir.dt.float32, tag="B")
            ident = cpool.tile([128, 128], mybir.dt.float32, tag="I")
            w_sb = cpool.tile([128, K, F], MM_DT, tag="W")
            bias_bc = cpool.tile([128, F], mybir.dt.float32, tag="bias")
            scale_col = cpool.tile([128, 1], mybir.dt.float32, tag="scale")
            eps_col = cpool.tile([128, 1], mybir.dt.float32, tag="eps")
            nc.vector.memset(eps_col[:], EPS)
            zeros128 = cpool.tile([128, 128], mybir.dt.float32, tag="zeros")
            nc.vector.memset(zeros128[:], 0.0)

            # identity/band consts first on the fast HWDGE queue (tiny, and
            # the very first transposes/win matmuls need them); W first on the
            # gpsimd queue with a casting DMA (fp32 -> float32r rounds, which
            # satisfies the verifier's fp32r-producer rule)
            nc.sync.dma_start(ident[:], I_d[:])
            nc.sync.dma_start(A_sb[:], A_d[:])
            nc.sync.dma_start(B_sb[:], B_d[:])
            # W in four chunks so batch-0's first mains track the arrival
            # (SWDGE casts fp32 -> float32r)
            for k0, k1 in ((0, 2), (2, 4), (4, 7), (7, 11), (11, K)):
                nc.gpsimd.dma_start(
                    w_sb[:, k0:k1, :], w_d[k0:k1].transpose([1, 0, 2])
                )
            nc.gpsimd.dma_start(
                bias_bc[:], bias_d[:].unsqueeze(0).partition_broadcast(128).squeeze(1)
            )
            nc.gpsimd.dma_start(
                scale_col[:],
                scale_d[:].flatten().unsqueeze(0).partition_broadcast(128).squeeze(1),
            )

            # Per-batch persistent tiles, created lazily by the prologue.
            xts = {}
            Ss = {}
            Rs = {}

            def load_group(b, i0, n, engine=None):
                """DMA one super-tile (n 128-row tiles, starting at tile i0)."""
                xn = xnpool.tile([128, n, 128], mybir.dt.float32, tag="xn")
                # src: x[b, i0*128 + i*128 + p, c] -> dest (p, i, c)
                src = x_d[b, i0 * 128 : (i0 + n) * 128, :].rearrange(
                    "(i p) c -> p i c", p=128
                )
                (engine or nc.sync).dma_start(xn[:], src)
                return xn

            def prologue_item(b, i, xn_group):
                """Square+transpose 128-row tile i of batch b from its super-tile."""
                if b not in xts:
                    xts[b] = xtpool.tile([128, XT_COLS], MM_DT, tag="xt", name=f"xt{b}")
                    Ss[b] = spool.tile([128, NT + 1], mybir.dt.float32, tag="S", name=f"S{b}")
                    Rs[b] = smpool.tile([128, NT], mybir.dt.float32, tag="R", name=f"R{b}")
                    nc.vector.tensor_copy(xts[b][:, L:XT_COLS], zeros128[:])
                    nc.vector.memset(Ss[b][:, NT : NT + 1], 0.0)
                xt, S = xts[b], Ss[b]
                if xn_group.ndim == 3:
                    xn_slice = xn_group[:, i % LG, :]
                else:
                    xn_slice = xn_group
                sq = sqpool.tile([128, 128], mybir.dt.float32, tag="sq")
                nc.scalar.activation(
                    sq[:],
                    xn_slice,
                    mybir.ActivationFunctionType.Square,
                    accum_out=S[:, i : i + 1],
                )
                pst = pstpool.tile([128, 128], mybir.dt.float32, tag="pst")
                nc.tensor.transpose(pst[:], xn_slice, ident[:])
                nc.vector.tensor_copy(xt[:, i * 128 : (i + 1) * 128], pst[:])

            def win_r(b, c0=0, c1=NT):
                """r = scale/sqrt(win+eps) for S columns [c0, c1) of batch b."""
                S, R = Ss[b], Rs[b]
                n = c1 - c0
                winp = pswpool.tile([128, n], mybir.dt.float32, tag="win", name=f"win{b}_{c0}")
                nc.tensor.matmul(winp[:], A_sb[:], S[:, c0:c1], start=True, stop=False)
                nc.tensor.matmul(
                    winp[:], B_sb[:], S[:, c0 + 1 : c1 + 1], start=False, stop=True
                )
                sd = smpool.tile([128, n], mybir.dt.float32, tag="sd", name=f"sd{b}_{c0}")
                nc.scalar.activation(
                    sd[:], winp[:], mybir.ActivationFunctionType.Sqrt, bias=eps_col[:]
                )
                rin = smpool.tile([128, n], mybir.dt.float32, tag="rin", name=f"rin{b}_{c0}")
                nc.vector.reciprocal(rin[:], sd[:])
                nc.vector.tensor_scalar_mul(R[:, c0:c1], rin[:], scale_col[:])

            def b0_head(b):
                """Batch-0 cold start: first 9 tiles with graduated load groups
                across both HWDGE queues, then r for the first 8 out-tiles."""
                sizes = [1, 1, 2, 4, 4]
                engines = [nc.scalar, nc.sync, nc.scalar, nc.sync, nc.scalar]
                i0 = 0
                for g, n in enumerate(sizes):
                    xn_group = load_group(b, i0, n, engines[g])
                    for i in range(i0, i0 + n):
                        prologue_item(b, i, xn_group[:, i - i0 : i - i0 + 1, :].squeeze(1))
                        if i == 8:
                            win_r(b, 0, 8)
                    i0 += n

            def b0_tail_item(b, i, xn_holder):
                """Items 9..31 of batch 0, in load groups of 4 on alternating
                queues; win chunks as soon as their S columns are complete.
                NOTE: every R slice must be emitted before any main that reads
                it -- Tile tracks deps by trace order only."""
                if i % 4 == 0:
                    eng = nc.sync if (i // 4) % 2 == 0 else nc.scalar
                    xn_holder[0] = load_group(b, i, 4, eng)
                    xn_holder[1] = i
                prologue_item(
                    b, i, xn_holder[0][:, i - xn_holder[1] : i - xn_holder[1] + 1, :].squeeze(1)
                )
                if i in (16, 24):
                    win_r(b, i - 8, i)
                elif i == NT - 1:
                    win_r(b, 24, NT)

            def main_tile(b, j, ob_group):
                """15 accumulated matmuls + evacuation for 128-row out tile j."""
                xt, R = xts[b], Rs[b]
                po = psopool.tile([128, F], mybir.dt.float32, tag="po")
                for k in range(K):
                    nc.tensor.matmul(
                        po[:],
                        xt[:, j * 128 + k : j * 128 + k + 128],
                        w_sb[:, k, :],
                        start=(k == 0),
                        stop=(k == K - 1),
                    )
                ob_slice = ob_group[:, j % SG, :]
                nc.scalar.activation(
                    ob_slice,
                    po[:],
                    mybir.ActivationFunctionType.Copy,
                    scale=R[:, j : j + 1],
                )
                nc.vector.tensor_add(ob_slice, ob_slice, bias_bc[:])

            def store_group(b, g, ob_group):
                """Store SG evacuated j-tiles as one DMA (split if partial)."""
                j0 = g * SG
                t0 = j0 * 128
                full = min(SG, (TOUT - t0 + 127) // 128)
                # dest iterated as (p, jj, f) to match src tile dims
                last_rows = min(128, TOUT - (j0 + full - 1) * 128)
                if last_rows == 128:
                    dst = out_d[b, t0 : t0 + full * 128, :].rearrange(
                        "(jj p) f -> p jj f", p=128
                    )
                    nc.scalar.dma_start(dst, ob_group[:, 0:full, :])
                else:
                    if full > 1:
                        dst = out_d[b, t0 : t0 + (full - 1) * 128, :].rearrange(
                            "(jj p) f -> p jj f", p=128
                        )
                        nc.scalar.dma_start(dst, ob_group[:, 0 : full - 1, :])
                    tl = (j0 + full - 1) * 128
                    nc.scalar.dma_start(
                        out_d[b, tl : tl + last_rows, :],
                        ob_group[:last_rows, full - 1, :],
                    )

            # ---- schedule ----
            b0_head(0)
            xn_holder = [None, 0]
            for b in range(BP):
                nxt = b + 1
                ob_group = None
                xn_cur = None
                for j in range(NT):
                    if b == 0 and j < 20:
                        # finish batch 0's own prologue under its mains;
                        # emitted BEFORE the main so win chunks precede readers
                        b0_tail_item(0, j + 12, xn_holder)
                    if j % SG == 0:
                        ob_group = respool.tile([128, SG, F], mybir.dt.float32, tag="ob")
                    main_tile(b, j, ob_group)
                    if j % SG == SG - 1:
                        store_group(b, j // SG, ob_group)
                    # interleave next batch's prologue into this batch's mains
                    if nxt < BP:
                        if j % LG == 0:
                            xn_cur = load_group(nxt, j, LG)
                        prologue_item(nxt, j, xn_cur)
                        if j == NT - 1:
                            win_r(nxt)

    nc.compile()
    return nc


def kernel(**inputs: np.ndarray) -> np.ndarray:
    global last_results
    x = np.ascontiguousarray(np.asarray(inputs["inputs"], dtype=np.float32))
    w = np.ascontiguousarray(np.asarray(inputs["kernel"], dtype=np.float32))
    scale = np.ascontiguousarray(np.asarray(inputs["scale"], dtype=np.float32))
    bias = np.ascontiguousarray(np.asarray(inputs["bias"], dtype=np.float32))

    if "nc" not in _cache:
        _cache["nc"] = _build()
    nc = _cache["nc"]

    in_maps = [
        {
            "x": np.ascontiguousarray(x[c * BP : (c + 1) * BP]),
            "w": w,
            "scale": scale,
            "bias": bias,
        }
        for c in range(NCORES)
    ]
    trace = os.environ.get("KERNEL_TRACE", "0") == "1"
    if "warm" not in _cache:
        # First execution on a cold device runs slow (model load, power
        # state); do an untraced warmup run so timed runs are steady-state.
        run_bass_kernel_spmd(nc, in_maps, core_ids=list(range(NCORES)), trace=False)
        _cache["warm"] = True
    res = run_bass_kernel_spmd(
        nc, in_maps, core_ids=list(range(NCORES)), trace=trace
    )
    last_results = res
    out = np.concatenate([res.results[c]["out"] for c in range(NCORES)], axis=0)
    return out


# revision 38
# speedup vs baseline: 1.0192x; 1.0034x over previous
"""Trainium2 Bass kernel for BioSphericalCKN1D (dense_cnn).

Computes, for x (32, 4096, 128), W (15, 128, 256), scale (1,1,1), bias (256):

    dot[b,t,f]  = sum_{k,c} x[b,t+k,c] * W[k,c,f]          (VALID conv, T_out = 4082)
    win[b,t]    = sum_{k,c} x[b,t+k,c]^2
    out[b,t,f]  = scale * dot / sqrt(win + 1e-7) + bias

Strategy:
  - Data-parallel over batch: 8 cores x 4 batches each; kernel/scale/bias replicated.
  - Per batch on-core:
      * load x in 1MB super-tiles, TensorE-transpose 128x128 tiles to xT [c, t]
      * ACT Square with fused row-sum accum -> per-t sum-of-squares S
      * sliding 15-window sums of S via 2 matmuls with constant band matrices
      * main conv: per 128-t tile, accumulate 15 float32r matmuls in PSUM
        (stationary = xT slice [c,128t], moving = W[k] [c,256f]) -> psum [t, f]
      * evacuate with ACT Copy scaled by r = scale/sqrt(win+eps) (per-partition),
        DVE add broadcast bias, batched contiguous DMA store.
  - Batch b+1's load/transpose/norm prologue is interleaved into batch b's
    main-matmul loop so the PE never idles (avoids HAM re-throttle).
"""

import os

import numpy as np

import concourse.bacc as bacc
import concourse.bass as bass
import concourse.mybir as mybir
import concourse.tile as tile
from concourse.bass_utils import run_bass_kernel_spmd

B, L, CIN, F, K = 32, 4096, 128, 256, 15
NCORES = 8
BP = B // NCORES          # batches per core
TOUT = L - K + 1          # 4082
NT = L // 128             # 32 row-tiles per batch
LG = 8                    # 128-row tiles per load super-tile (1 MB DMA)
SG = 4                    # j-tiles per store group (512 KB DMA)
EPS = 1e-7

# float32r = TF32-like PE mode: 1 cycle/row vs 4 for float32 (moving dim >= 256).
MM_DT = (
    mybir.dt.float32r
    if os.environ.get("KERNEL_MM_DT", "float32r") == "float32r"
    else mybir.dt.float32
)

_cache: dict = {}
last_results = None


def _build():
    nc = bacc.Bacc("TRN2", target_bir_lowering=False, debug=False, num_devices=NCORES)

    x_d = nc.dram_tensor("x", (BP, L, CIN), mybir.dt.float32, kind="ExternalInput")
    w_d = nc.dram_tensor("w", (K, CIN, F), mybir.dt.float32, kind="ExternalInput")
    scale_d = nc.dram_tensor("scale", (1, 1, 1), mybir.dt.float32, kind="ExternalInput")
    bias_d = nc.dram_tensor("bias", (F,), mybir.dt.float32, kind="ExternalInput")
    out_d = nc.dram_tensor("out", (BP, TOUT, F), mybir.dt.float32, kind="ExternalOutput")

    # Band matrices for the sliding-window sum: win[i*128+p] = sum_k sq[i*128+p+k]
    #   = (A.T @ S[:, i]) + (B.T @ S[:, i+1])   with A[q,p]=1 iff 0<=q-p<=K-1,
    #                                                B[q,p]=1 iff p-q>=128-(K-1)
    q = np.arange(128)[:, None]
    p = np.arange(128)[None, :]
    A_np = ((q - p >= 0) & (q - p <= K - 1)).astype(np.float32)
    B_np = (p - q >= 128 - (K - 1)).astype(np.float32)
    A_d = nc.inline_tensor(A_np, "bandA")
    B_d = nc.inline_tensor(B_np, "bandB")
    I_d = nc.inline_tensor(np.eye(128, dtype=np.float32), "ident")

    XT_COLS = L + 128  # main-mm lhsT slices read up to col 4109; zero-pad tail

    with tile.TileContext(nc) as tc:
        with (
            tc.tile_pool(name="const", bufs=1) as cpool,
            tc.tile_pool(name="xn", bufs=4) as xnpool,
            tc.tile_pool(name="sqs", bufs=3) as sqpool,
            tc.tile_pool(name="xt", bufs=2) as xtpool,
            tc.tile_pool(name="sums", bufs=2) as spool,
            tc.tile_pool(name="small", bufs=2) as smpool,
            tc.tile_pool(name="res", bufs=4) as respool,
            tc.tile_pool(name="pso", bufs=6, space=bass.MemorySpace.PSUM) as psopool,
            tc.tile_pool(name="pst", bufs=1, space=bass.MemorySpace.PSUM) as pstpool,
            tc.tile_pool(name="psw", bufs=1, space=bass.MemorySpace.PSUM) as pswpool,
        ):
            A_sb = cpool.tile([128, 128], mybir.dt.float32, tag="A")
            B_sb = cpool.tile([128, 128], mybir.dt.float32, tag="B")
            ident = cpool.tile([128, 128], mybir.dt.float32, tag="I")
            w_sb = cpool.tile([128, K, F], MM_DT, tag="W")
            bias_bc = cpool.tile([128, F], mybir.dt.float32, tag="bias")
            scale_col = cpool.tile([128, 1], mybir.dt.float32, tag="scale")
            eps_col = cpool.tile([128, 1], mybir.dt.float32, tag="eps")
            nc.vector.memset(eps_col[:], EPS)
            zeros128 = cpool.tile([128, 128], mybir.dt.float32, tag="zeros")
            nc.vector.memset(zeros128[:], 0.0)

            # identity/band consts first on the fast HWDGE queue (tiny, and
            # the very first transposes/win matmuls need them); W first on the
            # gpsimd queue with a casting DMA (fp32 -> float32r rounds, which
            # satisfies the verifier's fp32r-producer rule)
            nc.sync.dma_start(ident[:], I_d[:])
            nc.sync.dma_start(A_sb[:], A_d[:])
            nc.sync.dma_start(B_sb[:], B_d[:])
            # W in four chunks so batch-0's first mains track the arrival
            # (SWDGE casts fp32 -> float32r)
            for k0, k1 in ((0, 2), (2, 4), (4, 7), (7, 11), (11, K)):
                nc.gpsimd.dma_start(
                    w_sb[:, k0:k1, :], w_d[k0:k1].transpose([1, 0, 2])
                )
            nc.gpsimd.dma_start(
                bias_bc[:], bias_d[:].unsqueeze(0).partition_broadcast(128).squeeze(1)
            )
            nc.gpsimd.dma_start(
                scale_col[:],
                scale_d[:].flatten().unsqueeze(0).partition_broadcast(128).squeeze(1),
            )

            # Per-batch persistent tiles, created lazily by the prologue.
            xts = {}
            Ss = {}
            Rs = {}

            def load_group(b, i0, n, engine=None):
                """DMA one super-tile (n 128-row tiles, starting at tile i0)."""
                xn = xnpool.tile([128, n, 128], mybir.dt.float32, tag="xn")
                # src: x[b, i0*128 + i*128 + p, c] -> dest (p, i, c)
                src = x_d[b, i0 * 128 : (i0 + n) * 128, :].rearrange(
                    "(i p) c -> p i c", p=128
                )
                (engine or nc.sync).dma_start(xn[:], src)
                return xn

            def prologue_item(b, i, xn_group):
                """Square+transpose 128-row tile i of batch b from its super-tile."""
                if b not in xts:
                    xts[b] = xtpool.tile([128, XT_COLS], MM_DT, tag="xt", name=f"xt{b}")
                    Ss[b] = spool.tile([128, NT + 1], mybir.dt.float32, tag="S", name=f"S{b}")
                    Rs[b] = smpool.tile([128, NT], mybir.dt.float32, tag="R", name=f"R{b}")
                    nc.vector.tensor_copy(xts[b][:, L:XT_COLS], zeros128[:])
                    nc.vector.memset(Ss[b][:, NT : NT + 1], 0.0)
                xt, S = xts[b], Ss[b]
                if xn_group.ndim == 3:
                    xn_slice = xn_group[:, i % LG, :]
                else:
                    xn_slice = xn_group
                sq = sqpool.tile([128, 128], mybir.dt.float32, tag="sq")
                nc.scalar.activation(
                    sq[:],
                    xn_slice,
                    mybir.ActivationFunctionType.Square,
                    accum_out=S[:, i : i + 1],
                )
                pst = pstpool.tile([128, 128], mybir.dt.float32, tag="pst")
                nc.tensor.transpose(pst[:], xn_slice, ident[:])
                nc.vector.tensor_copy(xt[:, i * 128 : (i + 1) * 128], pst[:])

            def win_r(b, c0=0, c1=NT):
                """r = scale/sqrt(win+eps) for S columns [c0, c1) of batch b."""
                S, R = Ss[b], Rs[b]
                n = c1 - c0
                winp = pswpool.tile([128, n], mybir.dt.float32, tag="win", name=f"win{b}_{c0}")
                nc.tensor.matmul(winp[:], A_sb[:], S[:, c0:c1], start=True, stop=False)
                nc.tensor.matmul(
                    winp[:], B_sb[:], S[:, c0 + 1 : c1 + 1], start=False, stop=True
                )
                sd = smpool.tile([128, n], mybir.dt.float32, tag="sd", name=f"sd{b}_{c0}")
                nc.scalar.activation(
                    sd[:], winp[:], mybir.ActivationFunctionType.Sqrt, bias=eps_col[:]
                )
                rin = smpool.tile([128, n], mybir.dt.float32, tag="rin", name=f"rin{b}_{c0}")
                nc.vector.reciprocal(rin[:], sd[:])
                nc.vector.tensor_scalar_mul(R[:, c0:c1], rin[:], scale_col[:])

            def b0_head(b):
                """Batch-0 cold start: first 9 tiles with graduated load groups
                across both HWDGE queues, then r for the first 8 out-tiles."""
                sizes = [1, 1, 2, 4, 4, 4]
                engines = [nc.scalar, nc.sync, nc.scalar, nc.sync, nc.scalar, nc.sync]
                i0 = 0
                for g, n in enumerate(sizes):
                    xn_group = load_group(b, i0, n, engines[g])
                    for i in range(i0, i0 + n):
                        prologue_item(b, i, xn_group[:, i - i0 : i - i0 + 1, :].squeeze(1))
                        if i == 8:
                            win_r(b, 0, 8)
                    i0 += n

            def b0_tail_item(b, i, xn_holder):
                """Items 9..31 of batch 0, in load groups of 4 on alternating
                queues; win chunks as soon as their S columns are complete.
                NOTE: every R slice must be emitted before any main that reads
                it -- Tile tracks deps by trace order only."""
                if i % 4 == 0:
                    eng = nc.sync if (i // 4) % 2 == 0 else nc.scalar
                    xn_holder[0] = load_group(b, i, 4, eng)
                    xn_holder[1] = i
                prologue_item(
                    b, i, xn_holder[0][:, i - xn_holder[1] : i - xn_holder[1] + 1, :].squeeze(1)
                )
                if i in (16, 24):
                    win_r(b, i - 8, i)
                elif i == NT - 1:
                    win_r(b, 24, NT)

            def main_tile(b, j, ob_group):
                """15 accumulated matmuls + evacuation for 128-row out tile j."""
                xt, R = xts[b], Rs[b]
                po = psopool.tile([128, F], mybir.dt.float32, tag="po")
                for k in range(K):
                    nc.tensor.matmul(
                        po[:],
                        xt[:, j * 128 + k : j * 128 + k + 128],
                        w_sb[:, k, :],
                        start=(k == 0),
                        stop=(k == K - 1),
                    )
                ob_slice = ob_group[:, j % SG, :]
                nc.scalar.activation(
                    ob_slice,
                    po[:],
                    mybir.ActivationFunctionType.Copy,
                    scale=R[:, j : j + 1],
                )
                nc.vector.tensor_add(ob_slice, ob_slice, bias_bc[:])

            def store_group(b, g, ob_group):
                """Store SG evacuated j-tiles as one DMA (split if partial)."""
                j0 = g * SG
                t0 = j0 * 128
                full = min(SG, (TOUT - t0 + 127) // 128)
                # dest iterated as (p, jj, f) to match src tile dims
                last_rows = min(128, TOUT - (j0 + full - 1) * 128)
                if last_rows == 128:
                    dst = out_d[b, t0 : t0 + full * 128, :].rearrange(
                        "(jj p) f -> p jj f", p=128
                    )
                    nc.scalar.dma_start(dst, ob_group[:, 0:full, :])
                else:
                    if full > 1:
                        dst = out_d[b, t0 : t0 + (full - 1) * 128, :].rearrange(
                            "(jj p) f -> p jj f", p=128
                        )
                        nc.scalar.dma_start(dst, ob_group[:, 0 : full - 1, :])
                    tl = (j0 + full - 1) * 128
                    nc.scalar.dma_start(
                        out_d[b, tl : tl + last_rows, :],
                        ob_group[:last_rows, full - 1, :],
                    )

            # ---- schedule ----
            b0_head(0)
            xn_holder = [None, 0]
            for b in range(BP):
                nxt = b + 1
                ob_group = None
                xn_cur = None
                for j in range(NT):
                    if b == 0 and j < 16:
                        # finish batch 0's own prologue under its mains;
                        # emitted BEFORE the main so win chunks precede readers
                        b0_tail_item(0, j + 16, xn_holder)
                    if j % SG == 0:
                        ob_group = respool.tile([128, SG, F], mybir.dt.float32, tag="ob")
                    main_tile(b, j, ob_group)
                    if j % SG == SG - 1:
                        store_group(b, j // SG, ob_group)
                    # interleave next batch's prologue into this batch's mains
                    if nxt < BP:
                        if j % LG == 0:
                            xn_cur = load_group(nxt, j, LG)
                        prologue_item(nxt, j, xn_cur)
                        if j == NT - 1:
                            win_r(nxt)

    nc.compile()
    return nc


def kernel(**inputs: np.ndarray) -> np.ndarray:
    global last_results
    x = np.ascontiguousarray(np.asarray(inputs["inputs"], dtype=np.float32))
    w = np.ascontiguousarray(np.asarray(inputs["kernel"], dtype=np.float32))
    scale = np.ascontiguousarray(np.asarray(inputs["scale"], dtype=np.float32))
    bias = np.ascontiguousarray(np.asarray(inputs["bias"], dtype=np.float32))

    if "nc" not in _cache:
        _cache["nc"] = _build()
    nc = _cache["nc"]

    in_maps = [
        {
            "x": np.ascontiguousarray(x[c * BP : (c + 1) * BP]),
            "w": w,
            "scale": scale,
            "bias": bias,
        }
        for c in range(NCORES)
    ]
    trace = os.environ.get("KERNEL_TRACE", "0") == "1"
    if "warm" not in _cache:
        # First execution on a cold device runs slow (model load, power
        # state); do an untraced warmup run so timed runs are steady-state.
        run_bass_kernel_spmd(nc, in_maps, core_ids=list(range(NCORES)), trace=False)
        _cache["warm"] = True
    res = run_bass_kernel_spmd(
        nc, in_maps, core_ids=list(range(NCORES)), trace=trace
    )
    last_results = res
    out = np.concatenate([res.results[c]["out"] for c in range(NCORES)], axis=0)
    return out
